# revision 1
# baseline (speedup 1.0000x reference)
"""Trainium2 Bass kernel for nn_AfmoeAttention (GQA attention + gated output).

Sharding: 8 cores = 2 batches x 4 kv-groups. Each core handles one batch and
one kv head with its 8 query heads (tensor-parallel over heads, o_proj
row-parallel with the partial sums reduced on host during unsharding).

Per-core pipeline:
  A:  q/k/v/gate projections in one pass (hidden-stationary fp32r, weights
      moving as [q512 | k,v,g0 256 | g1 384] chunks), fused RMSNorm+RoPE
      read straight out of PSUM (gamma baked into host-prepared cos/sin
      tables; rsqrt on the DVE via bit-trick + Newton). q/k emitted in bf16
      and transposed to [d, s] via the DMA XBAR (no PE transposes). v kept
      [s, d+1] bf16 with a ones column; gate tanh on ACT straight from PSUM,
      sigmoid affine -> sg [s, f] bf16.
  B:  per chunk, per head pair, per key tile: scores^T = k^T q in bf16
      (one [128,1024] PSUM tile for both heads), exp on ScalarE
      (scale=D^-0.5, no max subtraction - scores are bounded) with a few
      key-tiles per pass routed to GpSimd instead (Schraudolph int16
      bit-trick exp -> bf16) to keep ACT below the PE roofline. PV uses
      P-stationary matmuls: lhsT = exp tile [keys, q128], moving = [v | 1]
      bf16 [keys, 65] -> PSUM accumulators [q, 65] per (head, q-subtile);
      col 64 accumulates the softmax denominator. Normalize via DVE
      reciprocal + per-partition scalar_tensor_tensor with sigmoid(gate),
      gated result transposed to [f, s] via the DMA XBAR for o_proj.
  C:  o_proj partial = gatedT^T @ WoT -> [s, HID] fp32r, summed on host.
      Interleaved into the NEXT chunk's key loop so the PE fills slack.
"""

import sys

import numpy as np

try:
    import concourse.bass as bass  # noqa: F401
except ImportError:
    sys.path.insert(0, "/opt/trn_rl_repo")

import concourse.mybir as mybir
import concourse.tile as tile
from concourse import bacc
from concourse.bass_utils import run_bass_kernel_spmd
from concourse.masks import make_identity  # noqa: F401  (identity kept for debug)


B, S, HID = 2, 2048, 2048
NH, NKV, D = 32, 4, 64
N_REP = NH // NKV            # 8 q-heads per kv head
EPS = 1e-6
SCALE = float(D) ** -0.5

P = 128
FP32 = mybir.dt.float32
FP32R = mybir.dt.float32r
BF16 = mybir.dt.bfloat16
I16 = mybir.dt.int16
AX = mybir.AxisListType.X
AF = mybir.ActivationFunctionType

# key tiles whose exp runs on GpSimd (int16 bit-trick) instead of ScalarE
POOL_TS = (0, 3, 6, 9, 13)
# int16 Schraudolph constants: i = score * (SCALE * 128*log2(e)) + B16,
# truncated to int16, bitcast bf16 ~= exp(score * SCALE) with ~2% ripple
A16 = 128.0 * 1.4426950408889634
B16 = 16256.0 - 5.5907


def _r(ap):
    return ap.bitcast(FP32R)


I32 = mybir.dt.int32
MAGIC = 0x5F3759DF
OP = mybir.AluOpType


def _rsqrt_dve(nc, stp, red, n, tag):
    """y = 1/sqrt(red/D + eps) on the DVE (bit-trick init + 2 Newton steps).

    Avoids ACT Sqrt so the whole kernel stays in the exp_and_others table set.
    """
    x = stp.tile([P, n], FP32, tag=tag + "x")
    nc.vector.tensor_scalar(out=x[:], in0=red[:], scalar1=1.0 / D, scalar2=EPS,
                            op0=OP.mult, op1=OP.add)
    y = stp.tile([P, n], FP32, tag=tag + "y")
    nc.vector.tensor_scalar(out=y[:].bitcast(I32), in0=x[:].bitcast(I32),
                            scalar1=1, scalar2=None, op0=OP.arith_shift_right)
    nc.vector.tensor_scalar(out=y[:].bitcast(I32), in0=y[:].bitcast(I32),
                            scalar1=MAGIC, scalar2=-1,
                            op0=OP.subtract, op1=OP.mult)
    h = stp.tile([P, n], FP32, tag=tag + "h")
    nc.vector.tensor_scalar(out=h[:], in0=red[:], scalar1=0.5 / D, scalar2=0.5 * EPS,
                            op0=OP.mult, op1=OP.add)
    t = stp.tile([P, n], FP32, tag=tag + "t")
    for _ in range(1):
        nc.vector.tensor_mul(t[:], y[:], y[:])
        nc.vector.tensor_mul(t[:], t[:], h[:])
        nc.vector.tensor_scalar(out=t[:], in0=t[:], scalar1=-1.0, scalar2=1.5,
                                op0=OP.mult, op1=OP.add)
        nc.vector.tensor_mul(y[:], y[:], t[:])
    return y


def build_program(s=S, hid=HID):
    """Build and bacc-compile the single-core SPMD program."""
    KK = hid // P            # contraction tiles over HID
    NI = s // P              # s-tiles
    SC = s // 512            # 512-wide s-chunks
    NC_HID = hid // 512      # o_proj output chunks
    PAIRS = N_REP // 2       # head pairs per core
    F = N_REP * D            # 512: per-core q/gate feature width
    NW = F + 2 * D + F       # 1152: [q 512 | k 64 | v 64 | g 512]
    H2 = D // 2

    nc = bacc.Bacc("TRN2", target_bir_lowering=False, debug=False,
                   enable_asserts=True, num_devices=1)

    ht_d = nc.dram_tensor("ht", [s // P, P, hid], BF16, kind="ExternalInput")
    w_d = nc.dram_tensor("w", [hid, NW], BF16, kind="ExternalInput")
    wot_d = nc.dram_tensor("wot", [F, hid], BF16, kind="ExternalInput")
    cq_d = nc.dram_tensor("cq", [s, D], FP32, kind="ExternalInput")
    sq_d = nc.dram_tensor("sq", [s, D], FP32, kind="ExternalInput")
    ck_d = nc.dram_tensor("ck", [s, D], FP32, kind="ExternalInput")
    sk_d = nc.dram_tensor("sk", [s, D], FP32, kind="ExternalInput")
    out_d = nc.dram_tensor("out", [s, hid], FP32, kind="ExternalOutput")

    w_v = w_d.ap().rearrange("(kk p) n -> p kk n", p=P)
    wot_v = wot_d.ap().rearrange("(ft p) n -> p ft n", p=P)

    with tile.TileContext(nc) as tc:
        with tc.tile_pool(name="pers", bufs=1) as pers:
            # persistent across phases
            qT2 = pers.tile([P, PAIRS, NI, P], BF16, tag="qT2")
            kT2 = pers.tile([P, NI, P], BF16, tag="kT2")   # [k | k dup] rows
            ve = pers.tile([P, NI, D + 1], BF16, tag="ve")  # [v | 1]
            sg = pers.tile([P, NI, F], BF16, tag="sg")      # sigmoid(gate)
            id32 = pers.tile([P, P], FP32, tag="id32")
            identr = pers.tile([P, P], FP32R, tag="identr")

            make_identity(nc, id32[:])
            nc.vector.tensor_copy(identr[:], id32[:])
            nc.gpsimd.memset(ve[:, :, D:D + 1], 1.0)

            # ---------------- Phase A: projections ----------------
            with tc.tile_pool(name="wq", bufs=1) as wqp, \
                 tc.tile_pool(name="ht", bufs=5) as htp, \
                 tc.tile_pool(name="cs", bufs=2) as csp, \
                 tc.tile_pool(name="scr", bufs=2) as scr, \
                 tc.tile_pool(name="stats", bufs=2) as stp, \
                 tc.tile_pool(name="psa", bufs=2, space="PSUM") as psa:

                w_sb = [None] * KK

                def get_w(kk):
                    if w_sb[kk] is None:
                        wt = wqp.tile([P, NW], BF16, name="wt", tag="w%d" % kk)
                        nc.sync.dma_start(wt[:], w_v[:, kk, :])
                        w_sb[kk] = wt
                    return w_sb[kk]

                htq = {}

                def load_htb(i):
                    # hT pre-tiled on host: [i-block, partition, h] gives 4KB
                    # contiguous bf16 DMA runs per partition
                    if i not in htq:
                        htb = htp.tile([P, KK, P], BF16, name="htb", tag="hta")
                        nc.sync.dma_start(htb[:], ht_d.ap()[i, :, :])
                        htq[i] = htb
                    return htq[i]

                def load_ht(i):
                    htb = load_htb(i)
                    pq = psa.tile([P, F], FP32, name="pq", tag="pq")
                    pkv = psa.tile([P, 256], FP32, name="pkv", tag="pkv")
                    pg1 = psa.tile([P, 384], FP32, name="pg1", tag="pg1")
                    return (htb, pq, pkv, pg1)

                def emit_mms(st, kk):
                    htb, pq, pkv, pg1 = st
                    hslc = htb[:, kk, :]
                    wt = get_w(kk)
                    nc.tensor.matmul(pq[:], hslc, wt[:, 0:F],
                                     start=(kk == 0), stop=(kk == KK - 1))
                    nc.tensor.matmul(pkv[:], hslc, wt[:, F:F + 256],
                                     start=(kk == 0), stop=(kk == KK - 1))
                    nc.tensor.matmul(pg1[:], hslc, wt[:, F + 256:NW],
                                     start=(kk == 0), stop=(kk == KK - 1))

                warm = {}
                for i in range(NI):
                    if i == 0:
                        # interleave the first two iterations' matmuls so the
                        # PE does 6 MMs (not 3) per weight-tile arrival during
                        # the DMA-paced warmup; ht + weight DMAs are queued
                        # before the (2MB of) rope tables so the first matmul
                        # isn't stuck behind table traffic
                        st0 = load_ht(0)
                        st1 = load_ht(1)
                        for kk in range(KK):
                            get_w(kk)
                        load_htb(2)   # prefetch ahead of the table traffic
                        cqa = csp.tile([P, NI, D], FP32, tag="cqa", bufs=1)
                        sqa = csp.tile([P, NI, D], FP32, tag="sqa", bufs=1)
                        cka = csp.tile([P, NI, D], FP32, tag="cka", bufs=1)
                        ska = csp.tile([P, NI, D], FP32, tag="ska", bufs=1)
                        cs_v = [t_d.ap().rearrange("(i p) d -> p i d", p=P)
                                for t_d in (cq_d, sq_d, ck_d, sk_d)]
                        nc.sync.dma_start(cqa[:], cs_v[0])
                        nc.sync.dma_start(sqa[:], cs_v[1])
                        nc.sync.dma_start(cka[:], cs_v[2])
                        nc.sync.dma_start(ska[:], cs_v[3])
                        load_htb(3)
                        load_htb(4)
                        for kk in range(KK):
                            emit_mms(st0, kk)
                            emit_mms(st1, kk)
                        warm[1] = st1
                        _, pq, pkv, pg1 = st0
                    elif i == 1:
                        _, pq, pkv, pg1 = warm.pop(1)
                    else:
                        st = load_ht(i)
                        for kk in range(KK):
                            emit_mms(st, kk)
                        _, pq, pkv, pg1 = st

                    cqt = cqa[:, i:i + 1, :]          # [P, 1, D]
                    sqt = sqa[:, i:i + 1, :]
                    ckt = cka[:, i, :]                # [P, D]
                    skt = ska[:, i, :]

                    # ---- q/k sum-of-squares on ACT (walrus allows only one
                    # PSUM read per vector op); k's reduction fused via accum
                    q3 = pq[:].rearrange("p (h d) -> p h d", d=D)
                    tsq = scr.tile([P, F], FP32, tag="tsq")
                    nc.scalar.square(tsq[:], pq[:])
                    red = stp.tile([P, N_REP + 1], FP32, tag="redq")
                    nc.vector.reduce_sum(red[:, 0:N_REP],
                                         tsq[:].rearrange("p (h d) -> p h d", d=D),
                                         axis=AX)
                    ktsq = scr.tile([P, D], FP32, tag="ktsq")
                    nc.scalar.activation(ktsq[:], pkv[:, 0:D], AF.Square,
                                         accum_out=red[:, N_REP:N_REP + 1])
                    rall = _rsqrt_dve(nc, stp, red, N_REP + 1, "rq")
                    rms2 = rall[:, 0:N_REP]
                    krms2 = rall[:, N_REP:N_REP + 1]

                    # ---- q rope + rms apply (bf16 out) + DMA-XBAR transpose
                    t2 = scr.tile([P, F], FP32, tag="t2")
                    t2v = t2[:].rearrange("p (h d) -> p h d", d=D)
                    nc.vector.tensor_mul(t2v[:, :, 0:H2], q3[:, :, H2:D],
                                         sqt[:, :, 0:H2].broadcast_to([P, N_REP, H2]))
                    nc.vector.tensor_mul(t2v[:, :, H2:D], q3[:, :, 0:H2],
                                         sqt[:, :, H2:D].broadcast_to([P, N_REP, H2]))
                    t3 = scr.tile([P, F], FP32, tag="t3")
                    t3v = t3[:].rearrange("p (h d) -> p h d", d=D)
                    nc.vector.tensor_mul(t3v, q3, cqt.broadcast_to([P, N_REP, D]))
                    nc.vector.tensor_add(t3[:], t3[:], t2[:])
                    t1 = scr.tile([P, F], BF16, tag="t1")
                    t1v = t1[:].rearrange("p (h d) -> p h d", d=D)
                    with nc.allow_low_precision(reason="bf16 q for bf16 QK matmul"):
                        nc.vector.tensor_mul(t1v, t3v, rms2[:, :, None].broadcast_to([P, N_REP, D]))
                    nc.sync.dma_start(qT2[:, :, i, :], t1[:], transpose=True)

                    # ---- k rope + rms apply, duplicated cols -> one transpose
                    kt2 = scr.tile([P, D], FP32, tag="kt2")
                    nc.vector.tensor_mul(kt2[:, 0:H2], pkv[:, H2:D], skt[:, 0:H2])
                    nc.vector.tensor_mul(kt2[:, H2:D], pkv[:, 0:H2], skt[:, H2:D])
                    kt3 = scr.tile([P, D], FP32, tag="kt3")
                    nc.vector.tensor_mul(kt3[:], pkv[:, 0:D], ckt[:])
                    nc.vector.tensor_add(kt3[:], kt3[:], kt2[:])
                    kt1 = scr.tile([P, P], BF16, tag="kt1")
                    kt1v = kt1[:].rearrange("p (two d) -> p two d", d=D)
                    with nc.allow_low_precision(reason="bf16 k for bf16 QK matmul"):
                        nc.vector.tensor_mul(
                            kt1v, kt3[:, None, :].broadcast_to([P, 2, D]),
                            krms2[:, :, None].broadcast_to([P, 2, D]))
                    nc.sync.dma_start(kT2[:, i, :], kt1[:], transpose=True)

                    # ---- v (bf16, ones col persistent)
                    nc.scalar.copy(ve[:, i, 0:D], pkv[:, D:2 * D])

                    # ---- gate: tanh from PSUM, sigmoid affine -> sg bf16
                    th = scr.tile([P, F], FP32, tag="th")
                    nc.scalar.activation(th[:, 0:P], pkv[:, P:256], AF.Tanh, scale=0.5)
                    nc.scalar.activation(th[:, P:F], pg1[:], AF.Tanh, scale=0.5)
                    with nc.allow_low_precision(reason="sigmoid affine to bf16 gate"):
                        nc.vector.tensor_scalar(out=sg[:, i, :], in0=th[:],
                                                scalar1=0.5, scalar2=0.5,
                                                op0=OP.mult, op1=OP.add)

            # ---------------- Phase B: attention (+C overlapped) ----------------
            with tc.tile_pool(name="psqk", bufs=2, space="PSUM") as psqk, \
                 tc.tile_pool(name="psat", bufs=1, space="PSUM") as psat, \
                 tc.tile_pool(name="psc", bufs=1, space="PSUM") as psc, \
                 tc.tile_pool(name="psc2", bufs=1, space="PSUM") as psc2, \
                 tc.tile_pool(name="expp", bufs=6) as expp, \
                 tc.tile_pool(name="misc", bufs=2) as mscp, \
                 tc.tile_pool(name="gst", bufs=4) as gsp, \
                 tc.tile_pool(name="wo", bufs=1) as wop:

                wot_sb = wop.tile([P, PAIRS, hid], BF16, tag="wot")
                nc.sync.dma_start(wot_sb[:], wot_v[:])

                cstate = {'flip': 0}

                def emit_c(gT, i, n, drain=False):
                    # one o_proj output tile [128, 512]; alternate two PSUM
                    # slots so back-to-back tiles pipeline during the drain
                    cstate['flip'] ^= 1
                    if cstate['flip']:
                        po = psc.tile([P, 512], FP32, name="po", tag="po")
                    else:
                        po = psc2.tile([P, 512], FP32, name="po2", tag="po2")
                    for ft in range(PAIRS):
                        nc.tensor.matmul(po[:], gT[:, ft, i % 4, :],
                                         wot_sb[:, ft, 512 * n:512 * (n + 1)],
                                         start=(ft == 0), stop=(ft == PAIRS - 1))
                    ob = mscp.tile([P, 512], FP32, name="ob", tag="ob", bufs=6)
                    if drain and cstate['flip']:
                        # ACT is idle during the drain: alternating the PSUM
                        # evacuation engines keeps both po slots cycling
                        nc.scalar.copy(ob[:], po[:])
                    else:
                        nc.vector.tensor_copy(ob[:], po[:])
                    nc.sync.dma_start(
                        out_d.ap()[P * i:P * (i + 1), 512 * n:512 * (n + 1)], ob[:])

                pending = []
                units = [(c, p, t) for c in range(SC)
                         for p in range(PAIRS) for t in range(NI)]
                ustate = {}
                pstate = {}

                def emit_qk(k):
                    c, p, t = units[k]
                    pq4 = psqk.tile([P, 1024], FP32, tag="pq4")
                    nc.tensor.matmul(pq4[:, 0:512], kT2[0:D, t, :],
                                     qT2[0:D, p, 4 * c:4 * (c + 1), :],
                                     start=True, stop=True)
                    nc.tensor.matmul(pq4[:, 512:1024], kT2[D:2 * D, t, :],
                                     qT2[D:2 * D, p, 4 * c:4 * (c + 1), :],
                                     start=True, stop=True)
                    ustate[k] = pq4

                def emit_exp(k):
                    c, p, t = units[k]
                    pq4 = ustate.pop(k)
                    if t in POOL_TS:
                        # Schraudolph exp on the DVE: int16 bit-trick
                        # (GPSIMD cannot read PSUM on hardware)
                        e16 = expp.tile([P, 1024], I16, tag="e16", bufs=6)
                        with nc.allow_low_precision(reason="approx exp bit-trick"):
                            nc.vector.tensor_scalar(
                                out=e16[:], in0=pq4[:],
                                scalar1=SCALE * A16, scalar2=B16,
                                op0=OP.mult, op1=OP.add)
                        ustate[k] = e16[:].bitcast(BF16)
                    else:
                        ex = expp.tile([P, 1024], BF16, tag="ee", bufs=6)
                        nc.scalar.activation(ex[:], pq4[:], AF.Exp, scale=SCALE)
                        ustate[k] = ex[:]

                def emit_pv(k):
                    c, p, t = units[k]
                    expE = ustate.pop(k)
                    if t == 0 and p == 0:
                        pstate['gstage'] = gsp.tile([P, 4, F], BF16, tag="gstage",
                                                    name="gstage")
                    if t == 0:
                        pstate['patA'] = psat.tile([P, 4, D + 1], FP32,
                                                   tag="patA", name="patA")
                        pstate['patB'] = psat.tile([P, 4, D + 1], FP32,
                                                   tag="patB", name="patB")
                    patA, patB = pstate['patA'], pstate['patB']
                    for h in range(2):
                        # one accumulation group per pat tile: PSUM start/stop
                        # is zero-region (2KB) granular, so the 4 q-subtile
                        # accumulators inside one tile share a single group
                        pat = patA if h == 0 else patB
                        for j in range(4):
                            nc.tensor.matmul(
                                pat[:, j, :],
                                expE[:, 512 * h + P * j:512 * h + P * (j + 1)],
                                ve[:, t, :],
                                start=(t == 0 and j == 0),
                                stop=(t == NI - 1 and j == 3),
                                skip_group_check=True)
                    if t < NI - 1:
                        return
                    # pair done: normalize by softmax denominator, gate, stage
                    gstage = pstate['gstage']
                    rr = mscp.tile([P, 8], FP32, tag="rr")
                    nc.vector.reciprocal(rr[:, 0:4], patA[:, :, D])
                    nc.vector.reciprocal(rr[:, 4:8], patB[:, :, D])
                    for h in range(2):
                        pat = patA if h == 0 else patB
                        for j in range(4):
                            fc = P * p + D * h
                            with nc.allow_low_precision(reason="gated attn bf16"):
                                nc.vector.scalar_tensor_tensor(
                                    out=gstage[:, j, fc:fc + D],
                                    in0=pat[:, j, 0:D],
                                    scalar=rr[:, 4 * h + j:4 * h + j + 1],
                                    in1=sg[:, 4 * c + j, fc:fc + D],
                                    op0=OP.mult, op1=OP.mult)
                    if p < PAIRS - 1:
                        return
                    # chunk done: transpose gated [s, f] -> [f, s] via the
                    # DMA XBAR; queue o_proj for the next chunk's slack
                    gT = gsp.tile([P, PAIRS, 4, P], BF16, tag="gT")
                    for j in range(4):
                        nc.sync.dma_start(gT[:, :, j, :], gstage[:, j, :],
                                          transpose=True)
                    pending.extend((gT, 4 * c + j, n)
                                   for j in range(4) for n in range(NC_HID))

                # three decoupled streams: QK runs one key-tile ahead of exp,
                # PV three behind, so neither ACT nor the PE wait-queue ever
                # blocks on an in-flight producer
                emit_qk(0)
                for k in range(len(units)):
                    emit_exp(k)
                    if k + 1 < len(units):
                        emit_qk(k + 1)
                    # o_proj burst AFTER the next QK so it never delays the
                    # exp stream's producer
                    if pending and units[k][2] in (0, 3, 6, 9):
                        emit_c(*pending.pop(0))
                    if k >= 4:
                        emit_pv(k - 4)
                for k in range(len(units) - 4, len(units)):
                    emit_pv(k)

                # drain the last chunk's o_proj over 4 rotating slots
                for (gT, i, n) in pending:
                    emit_c(gT, i, n, drain=True)

    nc.compile()
    return nc


def host_prep(hidden_states, cos, sin, Wq, Wk, Wv, Wg, Wo, q_gamma, k_gamma):
    """Shard and lay out the full inputs for the 8 cores (core = b*4 + g)."""
    import ml_dtypes
    bf = ml_dtypes.bfloat16
    f = N_REP * D
    in_maps = []
    s = hidden_states.shape[1]
    # tile[p, kk*128+c] for block i must equal hidden[b][128*i+c, kk*128+p]
    hT = []
    for b in range(B):
        x = np.asarray(hidden_states[b])
        t = x.reshape(s // P, P, HID // P, P)      # [i, c, kk, p]
        hT.append(np.ascontiguousarray(
            t.transpose(0, 3, 2, 1).reshape(s // P, P, HID)).astype(bf))
    # sign pattern of rotate_half and the (permuted) gamma baked into sin/cos
    sgn = np.concatenate([-np.ones(D // 2, np.float32), np.ones(D // 2, np.float32)])
    gq_perm = np.roll(q_gamma, -(D // 2))
    gk_perm = np.roll(k_gamma, -(D // 2))
    tabs = []
    for b in range(B):
        cq = np.ascontiguousarray(cos[b] * q_gamma[None, :]).astype(np.float32)
        sq = np.ascontiguousarray(sin[b] * (sgn * gq_perm)[None, :]).astype(np.float32)
        ck = np.ascontiguousarray(cos[b] * k_gamma[None, :]).astype(np.float32)
        sk2 = np.ascontiguousarray(sin[b] * (sgn * gk_perm)[None, :]).astype(np.float32)
        tabs.append((cq, sq, ck, sk2))
    for b in range(B):
        for g in range(NKV):
            wq = Wq[f * g:f * (g + 1), :].T               # [hid, 512]
            wk = Wk[D * g:D * (g + 1), :].T               # [hid, 64]
            wv = Wv[D * g:D * (g + 1), :].T               # [hid, 64]
            wg_ = Wg[f * g:f * (g + 1), :].T              # [hid, 512]
            w = np.ascontiguousarray(
                np.concatenate([wq, wk, wv, wg_], axis=1)).astype(bf)
            wot = np.ascontiguousarray(Wo[:, f * g:f * (g + 1)].T).astype(bf)
            cq, sq, ck, sk2 = tabs[b]
            in_maps.append(dict(ht=hT[b], w=w,
                                wot=wot, cq=cq, sq=sq, ck=ck, sk=sk2))
    return in_maps


_PROGRAM = None


def kernel(**inputs):
    global _PROGRAM
    if _PROGRAM is None:
        _PROGRAM = build_program()
    nc = _PROGRAM
    inputs = {k: np.asarray(v, dtype=np.float32) for k, v in inputs.items()}
    in_maps = host_prep(**inputs)
    res = run_bass_kernel_spmd(nc, in_maps, core_ids=list(range(8)))
    s, hid = inputs["hidden_states"].shape[1], inputs["hidden_states"].shape[2]
    out = np.zeros((B, s, hid), np.float32)
    for b in range(B):
        acc = np.zeros((s, hid), np.float64)
        for g in range(NKV):
            acc += res.results[b * NKV + g]["out"]
        out[b] = acc.astype(np.float32)
    return out



# revision 5
# speedup vs baseline: 1.0507x; 1.0507x over previous
"""Trainium2 Bass kernel for nn_AfmoeAttention (GQA attention + gated output).

Sharding: 8 cores = 2 batches x 4 kv-groups. Each core handles one batch and
one kv head with its 8 query heads (tensor-parallel over heads, o_proj
row-parallel with the partial sums reduced on host during unsharding).

Matmul strategy: fp8e4 DoubleRow (dual-fp8) everywhere it pays. Each value is
carried as an fp8 pair (value8, residual8), so precision matches bf16 while
the PE runs at 0.5 cycles/row with 2x contraction per instruction:

  A:  q/k/v/gate projections: host supplies h (x8) as an fp8 pair (h8,dh8)
      laid out [p, kk, 2, s] and W (x32) as fp8 w8 + dw8. Per 128-chunk kk:
      instr1 = (h8+dh8)^T w8 (pair=(h8,dh8), moving w8 broadcast over the
      pair dim), and per chunk-pair: instr2 = h8^T dw8 (pair over two kk).
      24 DoubleRow matmuls per i-tile instead of 48 bf16-equivalents.
      RMSNorm (scale-invariant, so the x256 wash out) + RoPE fused from
      PSUM; q/k emitted as [val8-as-bf16 | resid-bf16] staging, transposed
      via the bf16 DMA XBAR, then cast to fp8 packs post-transpose.
  B:  QK exact-fit: scores^T = (k8+dk8)^T (q8+dq8) in ONE DoubleRow matmul
      per (chunk, head, key-tile): lhsT = [k8;k8 | dk8;dk8] pair blocks,
      moving = [q8T; dq8T] stacked partitions broadcast over the pair dim.
      256 cycles per 128x512 score tile (2x the bf16 rate at bf16 accuracy).
      exp on ScalarE (a few key-tiles per pass on the DVE via Schraudolph
      int16 bit-trick). PV: P-stationary bf16 matmuls [keys,q128] x [v|1]
      -> PSUM [q, 65]; col 64 accumulates the softmax denominator.
      Normalize via DVE reciprocal + scalar_tensor_tensor with
      sigmoid(gate); gated result transposed to [f, s] via the DMA XBAR.
  C:  o_proj partial = gatedT^T @ WoT -> [s, HID] bf16, summed on host.
      Interleaved into the NEXT chunk's key loop so the PE fills slack.
"""

import sys

import numpy as np

try:
    import concourse.bass as bass  # noqa: F401
except ImportError:
    sys.path.insert(0, "/opt/trn_rl_repo")

import concourse.mybir as mybir
import concourse.tile as tile
from concourse import bacc
from concourse.bass_utils import run_bass_kernel_spmd
from concourse.masks import make_identity  # noqa: F401  (identity kept for debug)


B, S, HID = 2, 2048, 2048
NH, NKV, D = 32, 4, 64
N_REP = NH // NKV            # 8 q-heads per kv head
EPS = 1e-6
SCALE = float(D) ** -0.5

# host-side pre-scales (compensated on device; RMSNorm is scale-invariant)
H_SC = 8.0                   # hidden_states scale before fp8 pairing
W_SC = 32.0                  # qkvg weight scale before fp8 pairing
HW_SC = H_SC * W_SC          # projection outputs are x256
Q_SC = 8.0                   # q/k scale baked into rope tables (fp8 range)

P = 128
FP32 = mybir.dt.float32
FP8 = mybir.dt.float8e4
BF16 = mybir.dt.bfloat16
I16 = mybir.dt.int16
AX = mybir.AxisListType.X
AF = mybir.ActivationFunctionType
DR = mybir.MatmulPerfMode.DoubleRow

# key tiles whose exp runs on the DVE (int16 bit-trick) instead of ScalarE
POOL_TS = (0, 3, 6, 9, 13)
# int16 Schraudolph constants: i = score * (ESC * 128*log2(e)) + B16,
# truncated to int16, bitcast bf16 ~= exp(score * ESC) with ~2% ripple
ESC = SCALE / (Q_SC * Q_SC)  # scores in PSUM are x64
A16 = 128.0 * 1.4426950408889634
B16 = 16256.0 - 5.5907


I32 = mybir.dt.int32
MAGIC = 0x5F3759DF
OP = mybir.AluOpType


def _rsqrt_dve(nc, stp, red, n, tag):
    """y = 1/sqrt(red/D + eps) on the DVE (bit-trick init + 1 Newton step...
    plus a refinement); scale-invariant use: red is x65536, y comes out /256.
    """
    x = stp.tile([P, n], FP32, tag=tag + "x")
    nc.vector.tensor_scalar(out=x[:], in0=red[:], scalar1=1.0 / D, scalar2=EPS,
                            op0=OP.mult, op1=OP.add)
    y = stp.tile([P, n], FP32, tag=tag + "y")
    nc.vector.tensor_scalar(out=y[:].bitcast(I32), in0=x[:].bitcast(I32),
                            scalar1=1, scalar2=None, op0=OP.arith_shift_right)
    nc.vector.tensor_scalar(out=y[:].bitcast(I32), in0=y[:].bitcast(I32),
                            scalar1=MAGIC, scalar2=-1,
                            op0=OP.subtract, op1=OP.mult)
    h = stp.tile([P, n], FP32, tag=tag + "h")
    nc.vector.tensor_scalar(out=h[:], in0=red[:], scalar1=0.5 / D, scalar2=0.5 * EPS,
                            op0=OP.mult, op1=OP.add)
    t = stp.tile([P, n], FP32, tag=tag + "t")
    for _ in range(1):
        nc.vector.tensor_mul(t[:], y[:], y[:])
        nc.vector.tensor_mul(t[:], t[:], h[:])
        nc.vector.tensor_scalar(out=t[:], in0=t[:], scalar1=-1.0, scalar2=1.5,
                                op0=OP.mult, op1=OP.add)
        nc.vector.tensor_mul(y[:], y[:], t[:])
    return y


def build_program(s=S, hid=HID):
    """Build and bacc-compile the single-core SPMD program."""
    KK = hid // P            # contraction tiles over HID
    KJ = KK // 2             # kk-pairs for the dW residual pass
    NI = s // P              # s-tiles
    SC = s // 512            # 512-wide s-chunks
    NC_HID = hid // 512      # o_proj output chunks
    PAIRS = N_REP // 2       # head pairs (o_proj lhsT blocks)
    F = N_REP * D            # 512: per-core q/gate feature width
    NW = F + 2 * D + F       # 1152: [q 512 | k 64 | v 64 | g 512]
    H2 = D // 2

    nc = bacc.Bacc("TRN2", target_bir_lowering=False, debug=False,
                   enable_asserts=True, num_devices=1)

    ht_d = nc.dram_tensor("ht", [s // P, P, KK * 2 * P], FP8, kind="ExternalInput")
    w8_d = nc.dram_tensor("w8", [P, KK, NW], FP8, kind="ExternalInput")
    dw8_d = nc.dram_tensor("dw8", [P, KK, NW], FP8, kind="ExternalInput")
    wot_d = nc.dram_tensor("wot", [F, hid], BF16, kind="ExternalInput")
    cq_d = nc.dram_tensor("cq", [s, D], FP32, kind="ExternalInput")
    sq_d = nc.dram_tensor("sq", [s, D], FP32, kind="ExternalInput")
    ck_d = nc.dram_tensor("ck", [s, D], FP32, kind="ExternalInput")
    sk_d = nc.dram_tensor("sk", [s, D], FP32, kind="ExternalInput")
    out_d = nc.dram_tensor("out", [s, hid], FP32, kind="ExternalOutput")

    wot_v = wot_d.ap().rearrange("(ft p) n -> p ft n", p=P)

    with tile.TileContext(nc) as tc:
        with tc.tile_pool(name="pers", bufs=1) as pers:
            # persistent across phases
            qpk = pers.tile([P, N_REP, NI, P], FP8, tag="qpk")   # [q8T;dq8T] stacks
            kpk = pers.tile([P, NI, 2, P], FP8, tag="kpk")       # pair blocks
            ve = pers.tile([P, NI, D + 1], BF16, tag="ve")       # [v | 1]
            sg = pers.tile([P, NI, F], BF16, tag="sg")           # sigmoid(gate)

            # the ones column carries HW_SC so the softmax-denominator
            # reciprocal also cancels the x256 projection scale on v
            nc.gpsimd.memset(ve[:, :, D:D + 1], HW_SC)

            # ---------------- Phase A: projections ----------------
            with tc.tile_pool(name="wq", bufs=1) as wqp, \
                 tc.tile_pool(name="ht", bufs=5) as htp, \
                 tc.tile_pool(name="cs", bufs=2) as csp, \
                 tc.tile_pool(name="scr", bufs=2) as scr, \
                 tc.tile_pool(name="stg", bufs=2) as stg, \
                 tc.tile_pool(name="stats", bufs=2) as stp, \
                 tc.tile_pool(name="psa", bufs=2, space="PSUM") as psa:

                w_sb = [None] * KK
                dw_sb = [None] * KK

                def get_w(kk):
                    if w_sb[kk] is None:
                        wt = wqp.tile([P, NW], FP8, name="wt", tag="w%d" % kk)
                        nc.sync.dma_start(wt[:], w8_d.ap()[:, kk, :])
                        w_sb[kk] = wt
                    return w_sb[kk]

                def get_dw(kk):
                    # loaded pairwise so instr2's [kk, kk+1] slice is one tile
                    j = kk // 2
                    if dw_sb[j] is None:
                        wt = wqp.tile([P, 2, NW], FP8, name="dwt", tag="dw%d" % j)
                        nc.sync.dma_start(wt[:], dw8_d.ap()[:, 2 * j:2 * j + 2, :])
                        dw_sb[j] = wt
                    return dw_sb[j]

                htq = {}

                def load_htb(i):
                    if i not in htq:
                        htb = htp.tile([P, KK, 2, P], FP8, name="htb", tag="hta")
                        nc.sync.dma_start(
                            htb[:].rearrange("p a b c -> p (a b c)"),
                            ht_d.ap()[i, :, :])
                        htq[i] = htb
                    return htq[i]

                def load_ht(i):
                    htb = load_htb(i)
                    pq = psa.tile([P, F], FP32, name="pq", tag="pq")
                    pkv = psa.tile([P, 256], FP32, name="pkv", tag="pkv")
                    pg1 = psa.tile([P, 384], FP32, name="pg1", tag="pg1")
                    return (htb, pq, pkv, pg1)

                def emit_mms(st, kk):
                    # instr1: (h8+dh8)^T w8 ; pair dim on lhsT, w8 broadcast
                    htb, pq, pkv, pg1 = st
                    hp = htb[:, kk, :, :]
                    wt = get_w(kk)

                    def wbc(c0, c1):
                        return wt[:, c0:c1][:, None, :].broadcast_to(
                            [P, 2, c1 - c0])
                    nc.tensor.matmul(pq[:], hp, wbc(0, F),
                                     start=(kk == 0), stop=False, perf_mode=DR)
                    nc.tensor.matmul(pkv[:], hp, wbc(F, F + 256),
                                     start=(kk == 0), stop=False, perf_mode=DR)
                    nc.tensor.matmul(pg1[:], hp, wbc(F + 256, NW),
                                     start=(kk == 0), stop=False, perf_mode=DR)

                def emit_mms2(st, j):
                    # instr2: h8^T dw8 ; pair dim over (kk=2j, 2j+1)
                    htb, pq, pkv, pg1 = st
                    hp = htb[:, 2 * j:2 * j + 2, 0, :]
                    dwt = get_dw(2 * j)
                    last = (j == KJ - 1)
                    nc.tensor.matmul(pq[:], hp, dwt[:, :, 0:F],
                                     start=False, stop=last, perf_mode=DR)
                    nc.tensor.matmul(pkv[:], hp, dwt[:, :, F:F + 256],
                                     start=False, stop=last, perf_mode=DR)
                    nc.tensor.matmul(pg1[:], hp, dwt[:, :, F + 256:NW],
                                     start=False, stop=last, perf_mode=DR)

                def emit_all(st):
                    for kk in range(KK):
                        emit_mms(st, kk)
                    for j in range(KJ):
                        emit_mms2(st, j)

                warm = {}
                for i in range(NI):
                    if i == 0:
                        # interleave the first two iterations' matmuls so the
                        # PE does 6 MMs (not 3) per weight-tile arrival during
                        # the DMA-paced warmup; ht + weight DMAs are queued
                        # before the (2MB of) rope tables so the first matmul
                        # isn't stuck behind table traffic
                        st0 = load_ht(0)
                        st1 = load_ht(1)
                        for kk in range(KK):
                            get_w(kk)
                        load_htb(2)   # prefetch ahead of the table traffic
                        for kk in range(0, KK, 2):
                            get_dw(kk)
                        cqa = csp.tile([P, NI, D], FP32, tag="cqa", bufs=1)
                        sqa = csp.tile([P, NI, D], FP32, tag="sqa", bufs=1)
                        cka = csp.tile([P, NI, D], FP32, tag="cka", bufs=1)
                        ska = csp.tile([P, NI, D], FP32, tag="ska", bufs=1)
                        cs_v = [t_d.ap().rearrange("(i p) d -> p i d", p=P)
                                for t_d in (cq_d, sq_d, ck_d, sk_d)]
                        nc.sync.dma_start(cqa[:], cs_v[0])
                        nc.sync.dma_start(sqa[:], cs_v[1])
                        nc.sync.dma_start(cka[:], cs_v[2])
                        nc.sync.dma_start(ska[:], cs_v[3])
                        load_htb(3)
                        load_htb(4)
                        for kk in range(KK):
                            emit_mms(st0, kk)
                            emit_mms(st1, kk)
                        for j in range(KJ):
                            emit_mms2(st0, j)
                            emit_mms2(st1, j)
                        warm[1] = st1
                        _, pq, pkv, pg1 = st0
                    elif i == 1:
                        _, pq, pkv, pg1 = warm.pop(1)
                    else:
                        st = load_ht(i)
                        emit_all(st)
                        _, pq, pkv, pg1 = st

                    cqt = cqa[:, i:i + 1, :]          # [P, 1, D]
                    sqt = sqa[:, i:i + 1, :]
                    ckt = cka[:, i, :]                # [P, D]
                    skt = ska[:, i, :]

                    # ---- q/k sum-of-squares on ACT (scale washes out in rms)
                    q3 = pq[:].rearrange("p (h d) -> p h d", d=D)
                    tsq = scr.tile([P, F], FP32, tag="tsq")
                    nc.scalar.square(tsq[:], pq[:])
                    red = stp.tile([P, N_REP + 1], FP32, tag="redq")
                    nc.vector.reduce_sum(red[:, 0:N_REP],
                                         tsq[:].rearrange("p (h d) -> p h d", d=D),
                                         axis=AX)
                    ktsq = scr.tile([P, D], FP32, tag="ktsq")
                    nc.scalar.activation(ktsq[:], pkv[:, 0:D], AF.Square,
                                         accum_out=red[:, N_REP:N_REP + 1])
                    rall = _rsqrt_dve(nc, stp, red, N_REP + 1, "rq")
                    rms2 = rall[:, 0:N_REP]
                    krms2 = rall[:, N_REP:N_REP + 1]

                    # ---- q rope + rms apply -> fp8 pair staging -> XBAR
                    t2 = scr.tile([P, F], FP32, tag="t2")
                    t2v = t2[:].rearrange("p (h d) -> p h d", d=D)
                    nc.vector.tensor_mul(t2v[:, :, 0:H2], q3[:, :, H2:D],
                                         sqt[:, :, 0:H2].broadcast_to([P, N_REP, H2]))
                    nc.vector.tensor_mul(t2v[:, :, H2:D], q3[:, :, 0:H2],
                                         sqt[:, :, H2:D].broadcast_to([P, N_REP, H2]))
                    t3 = scr.tile([P, F], FP32, tag="t3")
                    t3v = t3[:].rearrange("p (h d) -> p h d", d=D)
                    nc.vector.tensor_mul(t3v, q3, cqt.broadcast_to([P, N_REP, D]))
                    nc.vector.tensor_add(t3[:], t3[:], t2[:])
                    t1f = scr.tile([P, F], FP32, tag="t1f")
                    t1v = t1f[:].rearrange("p (h d) -> p h d", d=D)
                    nc.vector.tensor_mul(t1v, t3v, rms2[:, :, None].broadcast_to([P, N_REP, D]))
                    with nc.allow_low_precision(reason="fp8 pair q for DoubleRow QK"):
                        q8t = stg.tile([P, F], FP8, tag="q8t")
                        nc.scalar.copy(q8t[:], t1f[:])
                        qsb = stg.tile([P, N_REP, 2, D], BF16, tag="qsb")
                        nc.scalar.copy(
                            qsb[:, :, 0, :],
                            q8t[:].rearrange("p (h d) -> p h d", d=D))
                        nc.vector.tensor_sub(
                            qsb[:, :, 1, :],
                            t1v, q8t[:].rearrange("p (h d) -> p h d", d=D))
                        qTs = stg.tile([P, N_REP, P], BF16, tag="qTs")
                        nc.sync.dma_start(
                            qTs[:], qsb[:].rearrange("p h a d -> p (h a d)"),
                            transpose=True)
                        nc.scalar.copy(qpk[:, :, i, :], qTs[:])

                    # ---- k rope + rms apply -> fp8 pair dup staging -> XBAR
                    kt2 = scr.tile([P, D], FP32, tag="kt2")
                    nc.vector.tensor_mul(kt2[:, 0:H2], pkv[:, H2:D], skt[:, 0:H2])
                    nc.vector.tensor_mul(kt2[:, H2:D], pkv[:, 0:H2], skt[:, H2:D])
                    kt3 = scr.tile([P, D], FP32, tag="kt3")
                    nc.vector.tensor_mul(kt3[:], pkv[:, 0:D], ckt[:])
                    nc.vector.tensor_add(kt3[:], kt3[:], kt2[:])
                    kf = scr.tile([P, D], FP32, tag="kf")
                    nc.vector.tensor_mul(kf[:], kt3[:],
                                         krms2[:].broadcast_to([P, D]))
                    with nc.allow_low_precision(reason="fp8 pair k for DoubleRow QK"):
                        k8t = stg.tile([P, D], FP8, tag="k8t")
                        nc.scalar.copy(k8t[:], kf[:])
                        dk = stg.tile([P, D], BF16, tag="dk")
                        nc.vector.tensor_sub(dk[:], kf[:], k8t[:])
                        ksb = stg.tile([P, 2, 2, D], BF16, tag="ksb")
                        nc.scalar.copy(ksb[:, 0, :, :],
                                       k8t[:, None, :].broadcast_to([P, 2, D]))
                        nc.scalar.copy(ksb[:, 1, :, :],
                                       dk[:, None, :].broadcast_to([P, 2, D]))
                        kTs = stg.tile([P, 2, P], BF16, tag="kTs")
                        nc.sync.dma_start(
                            kTs[:], ksb[:].rearrange("p a b d -> p (a b d)"),
                            transpose=True)
                        nc.scalar.copy(kpk[:, i, :, :], kTs[:])

                    # ---- v (bf16, ones col persistent; x256 cancels via
                    # the scaled denominator column)
                    nc.scalar.copy(ve[:, i, 0:D], pkv[:, D:2 * D])

                    # ---- gate: tanh from PSUM, sigmoid affine -> sg bf16
                    th = scr.tile([P, F], FP32, tag="th")
                    nc.scalar.activation(th[:, 0:P], pkv[:, P:256], AF.Tanh,
                                         scale=0.5 / HW_SC)
                    nc.scalar.activation(th[:, P:F], pg1[:], AF.Tanh,
                                         scale=0.5 / HW_SC)
                    with nc.allow_low_precision(reason="sigmoid affine to bf16 gate"):
                        nc.vector.tensor_scalar(out=sg[:, i, :], in0=th[:],
                                                scalar1=0.5, scalar2=0.5,
                                                op0=OP.mult, op1=OP.add)

            # ---------------- Phase B: attention (+C overlapped) ----------------
            with tc.tile_pool(name="psqk", bufs=3, space="PSUM") as psqk, \
                 tc.tile_pool(name="psat", bufs=1, space="PSUM") as psat, \
                 tc.tile_pool(name="psc", bufs=1, space="PSUM") as psc, \
                 tc.tile_pool(name="psc2", bufs=1, space="PSUM") as psc2, \
                 tc.tile_pool(name="expp", bufs=6) as expp, \
                 tc.tile_pool(name="misc", bufs=2) as mscp, \
                 tc.tile_pool(name="gst", bufs=4) as gsp, \
                 tc.tile_pool(name="wo", bufs=1) as wop:

                wot_sb = wop.tile([P, PAIRS, hid], BF16, tag="wot")
                nc.sync.dma_start(wot_sb[:], wot_v[:])

                qpk_v = qpk[:].rearrange("p h i s -> p h (i s)")

                cstate = {'flip': 0}

                def emit_c(gT, i, n, drain=False):
                    # one o_proj output tile [128, 512]; alternate two PSUM
                    # slots so back-to-back tiles pipeline during the drain
                    cstate['flip'] ^= 1
                    if cstate['flip']:
                        po = psc.tile([P, 512], FP32, name="po", tag="po")
                    else:
                        po = psc2.tile([P, 512], FP32, name="po2", tag="po2")
                    for ft in range(PAIRS):
                        nc.tensor.matmul(po[:], gT[:, ft, i % 4, :],
                                         wot_sb[:, ft, 512 * n:512 * (n + 1)],
                                         start=(ft == 0), stop=(ft == PAIRS - 1))
                    ob = mscp.tile([P, 512], FP32, name="ob", tag="ob", bufs=6)
                    if drain and cstate['flip']:
                        nc.scalar.copy(ob[:], po[:])
                    else:
                        nc.vector.tensor_copy(ob[:], po[:])
                    nc.sync.dma_start(
                        out_d.ap()[P * i:P * (i + 1), 512 * n:512 * (n + 1)], ob[:])

                pending = []
                units = [(c, h, t) for c in range(SC)
                         for h in range(N_REP) for t in range(NI)]
                ustate = {}
                pstate = {}

                def emit_qk(k):
                    c, h, t = units[k]
                    pq4 = psqk.tile([P, 512], FP32, tag="pq4")
                    rhs = qpk_v[:, h, 512 * c:512 * (c + 1)][:, None, :] \
                        .broadcast_to([P, 2, 512])
                    nc.tensor.matmul(pq4[:], kpk[:, t, :, :], rhs,
                                     start=True, stop=True, perf_mode=DR)
                    ustate[k] = pq4

                def emit_exp(k):
                    c, h, t = units[k]
                    pq4 = ustate.pop(k)
                    if t in POOL_TS:
                        # Schraudolph exp on the DVE: int16 bit-trick
                        e16 = expp.tile([P, 512], I16, tag="e16", bufs=6)
                        with nc.allow_low_precision(reason="approx exp bit-trick"):
                            nc.vector.tensor_scalar(
                                out=e16[:], in0=pq4[:],
                                scalar1=ESC * A16, scalar2=B16,
                                op0=OP.mult, op1=OP.add)
                        ustate[k] = e16[:].bitcast(BF16)
                    else:
                        ex = expp.tile([P, 512], BF16, tag="ee", bufs=6)
                        nc.scalar.activation(ex[:], pq4[:], AF.Exp, scale=ESC)
                        ustate[k] = ex[:]

                def emit_pv(k):
                    c, h, t = units[k]
                    expE = ustate.pop(k)
                    if t == 0 and h == 0:
                        pstate['gstage'] = gsp.tile([P, 4, F], BF16, tag="gstage",
                                                    name="gstage")
                    if t == 0:
                        pstate['pat'] = psat.tile([P, 4, D + 1], FP32,
                                                  tag="pat%d" % (h % 2),
                                                  name="pat")
                    pat = pstate['pat']
                    for j in range(4):
                        # one accumulation group per pat tile: PSUM start/stop
                        # is zero-region (2KB) granular, so the 4 q-subtile
                        # accumulators inside one tile share a single group
                        nc.tensor.matmul(
                            pat[:, j, :],
                            expE[:, P * j:P * (j + 1)],
                            ve[:, t, :],
                            start=(t == 0 and j == 0),
                            stop=(t == NI - 1 and j == 3),
                            skip_group_check=True)
                    if t < NI - 1:
                        return
                    # head done: normalize by softmax denominator, gate, stage
                    gstage = pstate['gstage']
                    rr = mscp.tile([P, 4], FP32, tag="rr")
                    nc.vector.reciprocal(rr[:], pat[:, :, D])
                    for j in range(4):
                        fc = D * h
                        with nc.allow_low_precision(reason="gated attn bf16"):
                            nc.vector.scalar_tensor_tensor(
                                out=gstage[:, j, fc:fc + D],
                                in0=pat[:, j, 0:D],
                                scalar=rr[:, j:j + 1],
                                in1=sg[:, 4 * c + j, fc:fc + D],
                                op0=OP.mult, op1=OP.mult)
                    if h < N_REP - 1:
                        return
                    # chunk done: transpose gated [s, f] -> [f, s] via the
                    # DMA XBAR; queue o_proj for the next chunk's slack
                    gT = gsp.tile([P, PAIRS, 4, P], BF16, tag="gT")
                    for j in range(4):
                        nc.sync.dma_start(gT[:, :, j, :], gstage[:, j, :],
                                          transpose=True)
                    pending.extend((gT, 4 * c + j, n)
                                   for j in range(4) for n in range(NC_HID))

                # three decoupled streams: QK runs one key-tile ahead of exp,
                # PV a few behind, so neither ACT nor the PE wait-queue ever
                # blocks on an in-flight producer
                emit_qk(0)
                for k in range(len(units)):
                    emit_exp(k)
                    if k + 1 < len(units):
                        emit_qk(k + 1)
                    # o_proj burst AFTER the next QK so it never delays the
                    # exp stream's producer
                    if pending and units[k][2] in (0, 3, 6, 9):
                        emit_c(*pending.pop(0))
                    if k >= 4:
                        emit_pv(k - 4)
                for k in range(len(units) - 4, len(units)):
                    emit_pv(k)

                # drain the last chunk's o_proj over rotating slots
                for (gT, i, n) in pending:
                    emit_c(gT, i, n, drain=True)

    nc.compile()
    return nc


def host_prep(hidden_states, cos, sin, Wq, Wk, Wv, Wg, Wo, q_gamma, k_gamma):
    """Shard and lay out the full inputs for the 8 cores (core = b*4 + g)."""
    import ml_dtypes
    bf = ml_dtypes.bfloat16
    f8 = ml_dtypes.float8_e4m3
    f = N_REP * D
    NW = f + 2 * D + f
    in_maps = []
    s = hidden_states.shape[1]
    KK = HID // P
    # hT pair-split: tile [i, p, kk, 2, c] with (h8, dh8), h scaled x8
    hT = []
    for b in range(B):
        x = np.asarray(hidden_states[b], dtype=np.float32) * H_SC
        t = x.reshape(s // P, P, KK, P)            # [i, c, kk, p]
        t = np.ascontiguousarray(t.transpose(0, 3, 2, 1))  # [i, p, kk, c]
        h8 = t.astype(f8)
        dh8 = (t - h8.astype(np.float32)).astype(f8)
        pk = np.stack([h8, dh8], axis=3)           # [i, p, kk, 2, c]
        hT.append(np.ascontiguousarray(pk.reshape(s // P, P, KK * 2 * P)))
    # sign pattern of rotate_half and the (permuted) gamma baked into sin/cos;
    # also the x8 fp8-range scale for q/k
    sgn = np.concatenate([-np.ones(D // 2, np.float32), np.ones(D // 2, np.float32)])
    gq_perm = np.roll(q_gamma, -(D // 2))
    gk_perm = np.roll(k_gamma, -(D // 2))
    tabs = []
    for b in range(B):
        cq = np.ascontiguousarray(cos[b] * q_gamma[None, :] * Q_SC).astype(np.float32)
        sq = np.ascontiguousarray(sin[b] * (sgn * gq_perm)[None, :] * Q_SC).astype(np.float32)
        ck = np.ascontiguousarray(cos[b] * k_gamma[None, :] * Q_SC).astype(np.float32)
        sk2 = np.ascontiguousarray(sin[b] * (sgn * gk_perm)[None, :] * Q_SC).astype(np.float32)
        tabs.append((cq, sq, ck, sk2))
    for b in range(B):
        for g in range(NKV):
            wq = Wq[f * g:f * (g + 1), :].T               # [hid, 512]
            wk = Wk[D * g:D * (g + 1), :].T               # [hid, 64]
            wv = Wv[D * g:D * (g + 1), :].T               # [hid, 64]
            wg_ = Wg[f * g:f * (g + 1), :].T              # [hid, 512]
            w = np.concatenate([wq, wk, wv, wg_], axis=1).astype(np.float32) * W_SC
            # [p, kk, nw] layout with hid split as (kk, p)
            wt = np.ascontiguousarray(
                w.reshape(KK, P, NW).transpose(1, 0, 2))
            w8 = wt.astype(f8)
            dw8 = (wt - w8.astype(np.float32)).astype(f8)
            wot = np.ascontiguousarray(Wo[:, f * g:f * (g + 1)].T).astype(bf)
            cq, sq, ck, sk2 = tabs[b]
            in_maps.append(dict(ht=hT[b], w8=np.ascontiguousarray(w8),
                                dw8=np.ascontiguousarray(dw8),
                                wot=wot, cq=cq, sq=sq, ck=ck, sk=sk2))
    return in_maps


_PROGRAM = None


def kernel(**inputs):
    global _PROGRAM
    if _PROGRAM is None:
        _PROGRAM = build_program()
    nc = _PROGRAM
    inputs = {k: np.asarray(v, dtype=np.float32) for k, v in inputs.items()}
    in_maps = host_prep(**inputs)
    res = run_bass_kernel_spmd(nc, in_maps, core_ids=list(range(8)))
    s, hid = inputs["hidden_states"].shape[1], inputs["hidden_states"].shape[2]
    out = np.zeros((B, s, hid), np.float32)
    for b in range(B):
        acc = np.zeros((s, hid), np.float64)
        for g in range(NKV):
            acc += res.results[b * NKV + g]["out"]
        out[b] = acc.astype(np.float32)
    return out


# revision 24
# speedup vs baseline: 1.0668x; 1.0153x over previous
"""Trainium2 Bass kernel for nn_AfmoeAttention (GQA attention + gated output).

Sharding: 8 cores = 2 batches x 4 kv-groups. Each core handles one batch and
one kv head with its 8 query heads (tensor-parallel over heads, o_proj
row-parallel with the partial sums reduced on host during unsharding).

Matmul strategy: fp8e4 DoubleRow (dual-fp8) everywhere it pays. Each value is
carried as an fp8 pair (value8, residual8), so precision matches bf16 while
the PE runs at 0.5 cycles/row with 2x contraction per instruction:

  A:  q/k/v/gate projections: host supplies h (x8) as an fp8 pair (h8,dh8)
      laid out [p, kk, 2, s] and W (x32) as fp8 w8 + dw8. Per 128-chunk kk:
      instr1 = (h8+dh8)^T w8 (pair=(h8,dh8), moving w8 broadcast over the
      pair dim), and per chunk-pair: instr2 = h8^T dw8 (pair over two kk).
      24 DoubleRow matmuls per i-tile instead of 48 bf16-equivalents.
      RMSNorm (scale-invariant, so the x256 wash out) + RoPE fused from
      PSUM; q/k emitted as [val8-as-bf16 | resid-bf16] staging, transposed
      via the bf16 DMA XBAR, then cast to fp8 packs post-transpose.
  B:  QK exact-fit: scores^T = (k8+dk8)^T (q8+dq8) in ONE DoubleRow matmul
      per (chunk, head, key-tile): lhsT = [k8;k8 | dk8;dk8] pair blocks,
      moving = [q8T; dq8T] stacked partitions broadcast over the pair dim.
      256 cycles per 128x512 score tile (2x the bf16 rate at bf16 accuracy).
      exp on ScalarE (a few key-tiles per pass on the DVE via Schraudolph
      int16 bit-trick). PV: P-stationary bf16 matmuls [keys,q128] x [v|1]
      -> PSUM [q, 65]; col 64 accumulates the softmax denominator.
      Normalize via DVE reciprocal + scalar_tensor_tensor with
      sigmoid(gate); gated result transposed to [f, s] via the DMA XBAR.
  C:  o_proj partial = gatedT^T @ WoT -> [s, HID] bf16, summed on host.
      Interleaved into the NEXT chunk's key loop so the PE fills slack.
"""

import sys

import numpy as np

try:
    import concourse.bass as bass  # noqa: F401
except ImportError:
    sys.path.insert(0, "/opt/trn_rl_repo")

import concourse.mybir as mybir
import concourse.tile as tile
from concourse import bacc
from concourse.bass_utils import run_bass_kernel_spmd
from concourse.masks import make_identity  # noqa: F401  (identity kept for debug)


B, S, HID = 2, 2048, 2048
NH, NKV, D = 32, 4, 64
N_REP = NH // NKV            # 8 q-heads per kv head
EPS = 1e-6
SCALE = float(D) ** -0.5

# host-side pre-scales (compensated on device; RMSNorm is scale-invariant)
H_SC = 8.0                   # hidden_states scale before fp8 pairing
W_SC = 32.0                  # qkvg weight scale before fp8 pairing
HW_SC = H_SC * W_SC          # projection outputs are x256
Q_SC = 8.0                   # q/k scale baked into rope tables (fp8 range)

P = 128
FP32 = mybir.dt.float32
FP8 = mybir.dt.float8e4
BF16 = mybir.dt.bfloat16
I16 = mybir.dt.int16
AX = mybir.AxisListType.X
AF = mybir.ActivationFunctionType
DR = mybir.MatmulPerfMode.DoubleRow

# exp alternates strictly between ScalarE (AF.Exp) and the DVE (Schraudolph
# int16 bit-trick): consecutive units always overlap on different engines
# int16 Schraudolph constants: i = score * (ESC * 128*log2(e)) + B16,
# truncated to int16, bitcast bf16 ~= exp(score * ESC) with ~2% ripple
ESC = SCALE / (Q_SC * Q_SC)  # scores in PSUM are x64
A16 = 128.0 * 1.4426950408889634
B16 = 16256.0 - 5.5907


I32 = mybir.dt.int32
MAGIC = 0x5F3759DF
OP = mybir.AluOpType


def _rsqrt_dve(nc, stp, red, n, tag):
    """y = 1/sqrt(red/D + eps) on the DVE (bit-trick init + 1 Newton step...
    plus a refinement); scale-invariant use: red is x65536, y comes out /256.
    """
    x = stp.tile([P, n], FP32, tag=tag + "x")
    nc.vector.tensor_scalar(out=x[:], in0=red[:], scalar1=1.0 / D, scalar2=EPS,
                            op0=OP.mult, op1=OP.add)
    y = stp.tile([P, n], FP32, tag=tag + "y")
    nc.vector.tensor_scalar(out=y[:].bitcast(I32), in0=x[:].bitcast(I32),
                            scalar1=1, scalar2=None, op0=OP.arith_shift_right)
    nc.vector.tensor_scalar(out=y[:].bitcast(I32), in0=y[:].bitcast(I32),
                            scalar1=MAGIC, scalar2=-1,
                            op0=OP.subtract, op1=OP.mult)
    h = stp.tile([P, n], FP32, tag=tag + "h")
    nc.vector.tensor_scalar(out=h[:], in0=red[:], scalar1=0.5 / D, scalar2=0.5 * EPS,
                            op0=OP.mult, op1=OP.add)
    t = stp.tile([P, n], FP32, tag=tag + "t")
    for _ in range(1):
        nc.vector.tensor_mul(t[:], y[:], y[:])
        nc.vector.tensor_mul(t[:], t[:], h[:])
        nc.vector.tensor_scalar(out=t[:], in0=t[:], scalar1=-1.0, scalar2=1.5,
                                op0=OP.mult, op1=OP.add)
        nc.vector.tensor_mul(y[:], y[:], t[:])
    return y


def build_program(s=S, hid=HID):
    """Build and bacc-compile the single-core SPMD program."""
    KK = hid // P            # contraction tiles over HID
    KJ = KK // 2             # kk-pairs for the dW residual pass
    NI = s // P              # s-tiles
    SC = s // 512            # 512-wide s-chunks
    NC_HID = hid // 512      # o_proj output chunks
    PAIRS = N_REP // 2       # head pairs (o_proj lhsT blocks)
    F = N_REP * D            # 512: per-core q/gate feature width
    NW = F + 2 * D + F       # 1152: [q 512 | k 64 | v 64 | g 512]
    H2 = D // 2

    nc = bacc.Bacc("TRN2", target_bir_lowering=False, debug=False,
                   enable_asserts=True, num_devices=1)

    ht_d = nc.dram_tensor("ht", [s // P, P, KK * 2 * P], FP8, kind="ExternalInput")
    w8_d = nc.dram_tensor("w8", [P, KK, NW], FP8, kind="ExternalInput")
    dw8_d = nc.dram_tensor("dw8", [P, KK, NW], FP8, kind="ExternalInput")
    wot_d = nc.dram_tensor("wot", [F, hid], BF16, kind="ExternalInput")
    cq_d = nc.dram_tensor("cq", [s, D], FP32, kind="ExternalInput")
    sq_d = nc.dram_tensor("sq", [s, D], FP32, kind="ExternalInput")
    ck_d = nc.dram_tensor("ck", [s, D], FP32, kind="ExternalInput")
    sk_d = nc.dram_tensor("sk", [s, D], FP32, kind="ExternalInput")
    out_d = nc.dram_tensor("out", [s, hid], FP32, kind="ExternalOutput")

    wot_v = wot_d.ap().rearrange("(ft p) n -> p ft n", p=P)

    with tile.TileContext(nc) as tc:
        with tc.tile_pool(name="pers", bufs=1) as pers:
            # persistent across phases
            qpk = pers.tile([P, N_REP, NI, P], FP8, tag="qpk")   # [q8T;dq8T] stacks
            kpk = pers.tile([P, NI, 2, P], FP8, tag="kpk")       # pair blocks
            ve = pers.tile([P, NI, D + 1], BF16, tag="ve")       # [v | 1]
            sg = pers.tile([P, NI, F], BF16, tag="sg")           # sigmoid(gate)

            # the ones column carries HW_SC so the softmax-denominator
            # reciprocal also cancels the x256 projection scale on v
            nc.gpsimd.memset(ve[:, :, D:D + 1], HW_SC)

            # ---------------- Phase A: projections ----------------
            with tc.tile_pool(name="wq", bufs=1) as wqp, \
                 tc.tile_pool(name="ht", bufs=5) as htp, \
                 tc.tile_pool(name="cs", bufs=2) as csp, \
                 tc.tile_pool(name="scr", bufs=2) as scr, \
                 tc.tile_pool(name="stg", bufs=2) as stg, \
                 tc.tile_pool(name="stats", bufs=2) as stp, \
                 tc.tile_pool(name="psa", bufs=2, space="PSUM") as psa:

                w_sb = [None] * KK
                dw_sb = [None] * KK

                def get_w(kk):
                    if w_sb[kk] is None:
                        wt = wqp.tile([P, NW], FP8, name="wt", tag="w%d" % kk)
                        nc.sync.dma_start(wt[:], w8_d.ap()[:, kk, :])
                        w_sb[kk] = wt
                    return w_sb[kk]

                def get_dw(kk):
                    # loaded pairwise so instr2's [kk, kk+1] slice is one tile
                    j = kk // 2
                    if dw_sb[j] is None:
                        wt = wqp.tile([P, 2, NW], FP8, name="dwt", tag="dw%d" % j)
                        nc.sync.dma_start(wt[:], dw8_d.ap()[:, 2 * j:2 * j + 2, :])
                        dw_sb[j] = wt
                    return dw_sb[j]

                htq = {}

                def load_htb(i):
                    if i not in htq:
                        htb = htp.tile([P, KK, 2, P], FP8, name="htb", tag="hta")
                        nc.sync.dma_start(
                            htb[:].rearrange("p a b c -> p (a b c)"),
                            ht_d.ap()[i, :, :])
                        htq[i] = htb
                    return htq[i]

                def load_ht(i):
                    htb = load_htb(i)
                    pq = psa.tile([P, F], FP32, name="pq", tag="pq")
                    pkv = psa.tile([P, 256], FP32, name="pkv", tag="pkv")
                    pg1 = psa.tile([P, 384], FP32, name="pg1", tag="pg1")
                    return (htb, pq, pkv, pg1)

                def emit_mms(st, kk):
                    # instr1: (h8+dh8)^T w8 ; pair dim on lhsT, w8 broadcast
                    htb, pq, pkv, pg1 = st
                    hp = htb[:, kk, :, :]
                    wt = get_w(kk)

                    def wbc(c0, c1):
                        return wt[:, c0:c1][:, None, :].broadcast_to(
                            [P, 2, c1 - c0])
                    nc.tensor.matmul(pq[:], hp, wbc(0, F),
                                     start=(kk == 0), stop=False, perf_mode=DR)
                    nc.tensor.matmul(pkv[:], hp, wbc(F, F + 256),
                                     start=(kk == 0), stop=False, perf_mode=DR)
                    nc.tensor.matmul(pg1[:], hp, wbc(F + 256, NW),
                                     start=(kk == 0), stop=False, perf_mode=DR)

                def emit_mms2(st, j):
                    # instr2: h8^T dw8 ; pair dim over (kk=2j, 2j+1)
                    htb, pq, pkv, pg1 = st
                    hp = htb[:, 2 * j:2 * j + 2, 0, :]
                    dwt = get_dw(2 * j)
                    last = (j == KJ - 1)
                    nc.tensor.matmul(pq[:], hp, dwt[:, :, 0:F],
                                     start=False, stop=last, perf_mode=DR)
                    nc.tensor.matmul(pkv[:], hp, dwt[:, :, F:F + 256],
                                     start=False, stop=last, perf_mode=DR)
                    nc.tensor.matmul(pg1[:], hp, dwt[:, :, F + 256:NW],
                                     start=False, stop=last, perf_mode=DR)

                def emit_all(st):
                    for kk in range(KK):
                        emit_mms(st, kk)
                    for j in range(KJ):
                        emit_mms2(st, j)

                warm = {}
                for i in range(NI):
                    if i == 0:
                        # interleave the first two iterations' matmuls so the
                        # PE does 6 MMs (not 3) per weight-tile arrival during
                        # the DMA-paced warmup; ht + weight DMAs are queued
                        # before the (2MB of) rope tables so the first matmul
                        # isn't stuck behind table traffic
                        st0 = load_ht(0)
                        st1 = load_ht(1)
                        for kk in range(KK):
                            get_w(kk)
                        load_htb(2)
                        cqa = csp.tile([P, NI, D], FP32, tag="cqa", bufs=1)
                        sqa = csp.tile([P, NI, D], FP32, tag="sqa", bufs=1)
                        cka = csp.tile([P, NI, D], FP32, tag="cka", bufs=1)
                        ska = csp.tile([P, NI, D], FP32, tag="ska", bufs=1)
                        cs_v = [t_d.ap().rearrange("(i p) d -> p i d", p=P)
                                for t_d in (cq_d, sq_d, ck_d, sk_d)]
                        nc.sync.dma_start(cqa[:], cs_v[0])
                        nc.sync.dma_start(sqa[:], cs_v[1])
                        nc.sync.dma_start(cka[:], cs_v[2])
                        nc.sync.dma_start(ska[:], cs_v[3])
                        for kk in range(0, KK, 2):
                            get_dw(kk)
                        load_htb(3)
                        load_htb(4)
                        for kk in range(KK):
                            emit_mms(st0, kk)
                            emit_mms(st1, kk)
                        for j in range(KJ):
                            emit_mms2(st0, j)
                            emit_mms2(st1, j)
                        warm[1] = st1
                        _, pq, pkv, pg1 = st0
                    elif i == 1:
                        _, pq, pkv, pg1 = warm.pop(1)
                    else:
                        st = load_ht(i)
                        emit_all(st)
                        _, pq, pkv, pg1 = st

                    cqt = cqa[:, i:i + 1, :]          # [P, 1, D]
                    sqt = sqa[:, i:i + 1, :]
                    ckt = cka[:, i, :]                # [P, D]
                    skt = ska[:, i, :]

                    # ---- q/k sum-of-squares on ACT (scale washes out in rms)
                    q3 = pq[:].rearrange("p (h d) -> p h d", d=D)
                    tsq = scr.tile([P, F], FP32, tag="tsq")
                    nc.scalar.square(tsq[:], pq[:])
                    red = stp.tile([P, N_REP + 1], FP32, tag="redq")
                    nc.vector.reduce_sum(red[:, 0:N_REP],
                                         tsq[:].rearrange("p (h d) -> p h d", d=D),
                                         axis=AX)
                    ktsq = scr.tile([P, D], FP32, tag="ktsq")
                    nc.scalar.activation(ktsq[:], pkv[:, 0:D], AF.Square,
                                         accum_out=red[:, N_REP:N_REP + 1])
                    rall = _rsqrt_dve(nc, stp, red, N_REP + 1, "rq")
                    rms2 = rall[:, 0:N_REP]
                    krms2 = rall[:, N_REP:N_REP + 1]

                    # ---- q rope + rms apply -> fp8 pair staging -> XBAR
                    t2 = scr.tile([P, F], FP32, tag="t2")
                    t2v = t2[:].rearrange("p (h d) -> p h d", d=D)
                    nc.vector.tensor_mul(t2v[:, :, 0:H2], q3[:, :, H2:D],
                                         sqt[:, :, 0:H2].broadcast_to([P, N_REP, H2]))
                    nc.vector.tensor_mul(t2v[:, :, H2:D], q3[:, :, 0:H2],
                                         sqt[:, :, H2:D].broadcast_to([P, N_REP, H2]))
                    t3 = scr.tile([P, F], FP32, tag="t3")
                    t3v = t3[:].rearrange("p (h d) -> p h d", d=D)
                    nc.vector.tensor_mul(t3v, q3, cqt.broadcast_to([P, N_REP, D]))
                    nc.vector.tensor_add(t3[:], t3[:], t2[:])
                    t1f = scr.tile([P, F], FP32, tag="t1f")
                    t1v = t1f[:].rearrange("p (h d) -> p h d", d=D)
                    nc.vector.tensor_mul(t1v, t3v, rms2[:, :, None].broadcast_to([P, N_REP, D]))
                    with nc.allow_low_precision(reason="fp8 pair q for DoubleRow QK"):
                        q8t = stg.tile([P, F], FP8, tag="q8t")
                        nc.vector.tensor_copy(q8t[:], t1f[:])
                        qsb = stg.tile([P, N_REP, 2, D], BF16, tag="qsb")
                        nc.gpsimd.tensor_copy(
                            qsb[:, :, 0, :],
                            q8t[:].rearrange("p (h d) -> p h d", d=D))
                        nc.vector.tensor_sub(
                            qsb[:, :, 1, :],
                            t1v, q8t[:].rearrange("p (h d) -> p h d", d=D))
                        qTs = stg.tile([P, N_REP, P], BF16, tag="qTs")
                        nc.sync.dma_start(
                            qTs[:], qsb[:].rearrange("p h a d -> p (h a d)"),
                            transpose=True)
                        nc.gpsimd.tensor_copy(qpk[:, :, i, :], qTs[:])

                    # ---- k rope + rms apply -> fp8 pair dup staging -> XBAR
                    kt2 = scr.tile([P, D], FP32, tag="kt2")
                    nc.vector.tensor_mul(kt2[:, 0:H2], pkv[:, H2:D], skt[:, 0:H2])
                    nc.vector.tensor_mul(kt2[:, H2:D], pkv[:, 0:H2], skt[:, H2:D])
                    kt3 = scr.tile([P, D], FP32, tag="kt3")
                    nc.vector.tensor_mul(kt3[:], pkv[:, 0:D], ckt[:])
                    nc.vector.tensor_add(kt3[:], kt3[:], kt2[:])
                    kf = scr.tile([P, D], FP32, tag="kf")
                    nc.vector.tensor_mul(kf[:], kt3[:],
                                         krms2[:].broadcast_to([P, D]))
                    with nc.allow_low_precision(reason="fp8 pair k for DoubleRow QK"):
                        k8t = stg.tile([P, D], FP8, tag="k8t")
                        nc.scalar.copy(k8t[:], kf[:])
                        dk = stg.tile([P, D], BF16, tag="dk")
                        nc.vector.tensor_sub(dk[:], kf[:], k8t[:])
                        ksb = stg.tile([P, 2, 2, D], BF16, tag="ksb")
                        nc.gpsimd.tensor_copy(ksb[:, 0, :, :],
                                              k8t[:, None, :].broadcast_to([P, 2, D]))
                        nc.gpsimd.tensor_copy(ksb[:, 1, :, :],
                                              dk[:, None, :].broadcast_to([P, 2, D]))
                        kTs = stg.tile([P, 2, P], BF16, tag="kTs")
                        nc.sync.dma_start(
                            kTs[:], ksb[:].rearrange("p a b d -> p (a b d)"),
                            transpose=True)
                        nc.gpsimd.tensor_copy(kpk[:, i, :, :], kTs[:])

                    # ---- v (bf16, ones col persistent; x256 cancels via
                    # the scaled denominator column)
                    nc.scalar.copy(ve[:, i, 0:D], pkv[:, D:2 * D])

                    # ---- gate: tanh from PSUM, sigmoid affine -> sg bf16
                    th = scr.tile([P, F], FP32, tag="th")
                    nc.scalar.activation(th[:, 0:P], pkv[:, P:256], AF.Tanh,
                                         scale=0.5 / HW_SC)
                    nc.scalar.activation(th[:, P:F], pg1[:], AF.Tanh,
                                         scale=0.5 / HW_SC)
                    with nc.allow_low_precision(reason="sigmoid affine to bf16 gate"):
                        nc.vector.tensor_scalar(out=sg[:, i, :], in0=th[:],
                                                scalar1=0.5, scalar2=0.5,
                                                op0=OP.mult, op1=OP.add)

            # ---------------- Phase B: attention (+C overlapped) ----------------
            with tc.tile_pool(name="psqk", bufs=3, space="PSUM") as psqk, \
                 tc.tile_pool(name="psat", bufs=1, space="PSUM") as psat, \
                 tc.tile_pool(name="psc", bufs=1, space="PSUM") as psc, \
                 tc.tile_pool(name="expp", bufs=6) as expp, \
                 tc.tile_pool(name="misc", bufs=2) as mscp, \
                 tc.tile_pool(name="gst", bufs=4) as gsp, \
                 tc.tile_pool(name="wo", bufs=1) as wop:

                wot_sb = wop.tile([P, PAIRS, hid], BF16, tag="wot")
                nc.sync.dma_start(wot_sb[:], wot_v[:])

                qpk_v = qpk[:].rearrange("p h i s -> p h (i s)")

                def emit_c(gT, i, n, drain=False):
                    # one o_proj output tile [128, 512]; single PSUM slot in
                    # steady state (filler work), psqk's freed slots for the
                    # drain so back-to-back tiles pipeline there
                    if drain:
                        po = psqk.tile([P, 2, 512], FP32, name="pod",
                                       tag="pq4")[:, 0, :]
                    else:
                        po = psc.tile([P, 512], FP32, name="po", tag="po")
                    for ft in range(PAIRS):
                        nc.tensor.matmul(po[:], gT[:, ft, i % 4, :],
                                         wot_sb[:, ft, 512 * n:512 * (n + 1)],
                                         start=(ft == 0), stop=(ft == PAIRS - 1))
                    ob = mscp.tile([P, 512], FP32, name="ob", tag="ob", bufs=6)
                    nc.scalar.copy(ob[:], po[:])
                    nc.sync.dma_start(
                        out_d.ap()[P * i:P * (i + 1), 512 * n:512 * (n + 1)], ob[:])

                pending = []
                NTP = NI // 2
                units = [(c, h, tp) for c in range(SC)
                         for h in range(N_REP) for tp in range(NTP)]
                ustate = {}
                pstate = {}

                def emit_qk(k):
                    c, h, tp = units[k]
                    pq4 = psqk.tile([P, 2, 512], FP32, tag="pq4")
                    rhs = qpk_v[:, h, 512 * c:512 * (c + 1)][:, None, :] \
                        .broadcast_to([P, 2, 512])
                    for dt in range(2):
                        nc.tensor.matmul(pq4[:, dt, :], kpk[:, 2 * tp + dt, :, :],
                                         rhs, start=True, stop=True, perf_mode=DR)
                    ustate[k] = pq4

                def emit_exp(k):
                    c, h, tp = units[k]
                    pq4 = ustate.pop(k)
                    pqf = pq4[:].rearrange("p a b -> p (a b)")
                    if k % 2:
                        # Schraudolph exp on the DVE: int16 bit-trick
                        e16 = expp.tile([P, 1024], I16, tag="e16", bufs=6)
                        with nc.allow_low_precision(reason="approx exp bit-trick"):
                            nc.vector.tensor_scalar(
                                out=e16[:], in0=pqf,
                                scalar1=ESC * A16, scalar2=B16,
                                op0=OP.mult, op1=OP.add)
                        ustate[k] = e16[:].bitcast(BF16).rearrange(
                            "p (a b) -> p a b", a=2)
                    else:
                        ex = expp.tile([P, 2, 512], BF16, tag="ee", bufs=6)
                        nc.scalar.activation(ex[:].rearrange("p a b -> p (a b)"),
                                             pqf, AF.Exp, scale=ESC)
                        ustate[k] = ex[:]

                def emit_pv(k):
                    c, h, tp = units[k]
                    expE = ustate.pop(k)
                    if tp == 0 and h == 0:
                        pstate['gstage'] = gsp.tile([P, 4, F], BF16, tag="gstage",
                                                    name="gstage")
                    if tp == 0:
                        # single tag: the pool's WAR dependency on the gating
                        # read of the previous head orders reuse correctly
                        pstate['pat'] = psat.tile([P, 4, D + 1], FP32,
                                                  tag="pat", name="pat")
                    pat = pstate['pat']
                    for dt in range(2):
                        for j in range(4):
                            # one accumulation group per pat tile: PSUM
                            # start/stop is zero-region (2KB) granular, so the
                            # 4 q-subtile accumulators share a single group
                            nc.tensor.matmul(
                                pat[:, j, :],
                                expE[:, dt, P * j:P * (j + 1)],
                                ve[:, 2 * tp + dt, :],
                                start=(tp == 0 and dt == 0 and j == 0),
                                stop=(tp == NTP - 1 and dt == 1 and j == 3),
                                skip_group_check=True)
                    if tp < NTP - 1:
                        return
                    # head done: normalize by softmax denominator, gate, stage
                    gstage = pstate['gstage']
                    rr = mscp.tile([P, 4], FP32, tag="rr")
                    nc.vector.reciprocal(rr[:], pat[:, :, D])
                    for j in range(4):
                        fc = D * h
                        with nc.allow_low_precision(reason="gated attn bf16"):
                            nc.vector.scalar_tensor_tensor(
                                out=gstage[:, j, fc:fc + D],
                                in0=pat[:, j, 0:D],
                                scalar=rr[:, j:j + 1],
                                in1=sg[:, 4 * c + j, fc:fc + D],
                                op0=OP.mult, op1=OP.mult)
                    if h < N_REP - 1:
                        return
                    # chunk done: transpose gated [s, f] -> [f, s] via the
                    # DMA XBAR; queue o_proj for the next chunk's slack
                    gT = gsp.tile([P, PAIRS, 4, P], BF16, tag="gT")
                    for j in range(4):
                        nc.sync.dma_start(gT[:, :, j, :], gstage[:, j, :],
                                          transpose=True)
                    pending.extend((gT, 4 * c + j, n)
                                   for j in range(4) for n in range(NC_HID))

                # three decoupled streams: QK runs one pair-unit ahead of exp,
                # PV two behind, so neither ACT nor the PE wait-queue ever
                # blocks on an in-flight producer
                emit_qk(0)
                for k in range(len(units)):
                    emit_exp(k)
                    if k + 1 < len(units):
                        emit_qk(k + 1)
                    # o_proj AFTER the next QK so it never delays the exp
                    # stream's producer; 16 pop slots per chunk exactly match
                    # the 16 queued tiles, and tp==0/4 placement gives the PE
                    # filler work while the DVE runs the previous head's
                    # gating tail (pat WAR would otherwise idle the PE)
                    if pending and units[k][2] in (0, 4):
                        emit_c(*pending.pop(0))
                    if k >= 2:
                        emit_pv(k - 2)
                for k in range(len(units) - 2, len(units)):
                    emit_pv(k)

                # drain the last chunk's o_proj over rotating slots
                for (gT, i, n) in pending:
                    emit_c(gT, i, n, drain=True)

    nc.compile()
    return nc


def host_prep(hidden_states, cos, sin, Wq, Wk, Wv, Wg, Wo, q_gamma, k_gamma):
    """Shard and lay out the full inputs for the 8 cores (core = b*4 + g)."""
    import ml_dtypes
    bf = ml_dtypes.bfloat16
    f8 = ml_dtypes.float8_e4m3
    f = N_REP * D
    NW = f + 2 * D + f
    in_maps = []
    s = hidden_states.shape[1]
    KK = HID // P
    # hT pair-split: tile [i, p, kk, 2, c] with (h8, dh8), h scaled x8
    hT = []
    for b in range(B):
        x = np.asarray(hidden_states[b], dtype=np.float32) * H_SC
        t = x.reshape(s // P, P, KK, P)            # [i, c, kk, p]
        t = np.ascontiguousarray(t.transpose(0, 3, 2, 1))  # [i, p, kk, c]
        h8 = t.astype(f8)
        dh8 = (t - h8.astype(np.float32)).astype(f8)
        pk = np.stack([h8, dh8], axis=3)           # [i, p, kk, 2, c]
        hT.append(np.ascontiguousarray(pk.reshape(s // P, P, KK * 2 * P)))
    # sign pattern of rotate_half and the (permuted) gamma baked into sin/cos;
    # also the x8 fp8-range scale for q/k
    sgn = np.concatenate([-np.ones(D // 2, np.float32), np.ones(D // 2, np.float32)])
    gq_perm = np.roll(q_gamma, -(D // 2))
    gk_perm = np.roll(k_gamma, -(D // 2))
    tabs = []
    for b in range(B):
        cq = np.ascontiguousarray(cos[b] * q_gamma[None, :] * Q_SC).astype(np.float32)
        sq = np.ascontiguousarray(sin[b] * (sgn * gq_perm)[None, :] * Q_SC).astype(np.float32)
        ck = np.ascontiguousarray(cos[b] * k_gamma[None, :] * Q_SC).astype(np.float32)
        sk2 = np.ascontiguousarray(sin[b] * (sgn * gk_perm)[None, :] * Q_SC).astype(np.float32)
        tabs.append((cq, sq, ck, sk2))
    for b in range(B):
        for g in range(NKV):
            wq = Wq[f * g:f * (g + 1), :].T               # [hid, 512]
            wk = Wk[D * g:D * (g + 1), :].T               # [hid, 64]
            wv = Wv[D * g:D * (g + 1), :].T               # [hid, 64]
            wg_ = Wg[f * g:f * (g + 1), :].T              # [hid, 512]
            w = np.concatenate([wq, wk, wv, wg_], axis=1).astype(np.float32) * W_SC
            # [p, kk, nw] layout with hid split as (kk, p)
            wt = np.ascontiguousarray(
                w.reshape(KK, P, NW).transpose(1, 0, 2))
            w8 = wt.astype(f8)
            dw8 = (wt - w8.astype(np.float32)).astype(f8)
            wot = np.ascontiguousarray(Wo[:, f * g:f * (g + 1)].T).astype(bf)
            cq, sq, ck, sk2 = tabs[b]
            in_maps.append(dict(ht=hT[b], w8=np.ascontiguousarray(w8),
                                dw8=np.ascontiguousarray(dw8),
                                wot=wot, cq=cq, sq=sq, ck=ck, sk=sk2))
    return in_maps


_PROGRAM = None


def kernel(**inputs):
    global _PROGRAM
    if _PROGRAM is None:
        _PROGRAM = build_program()
    nc = _PROGRAM
    inputs = {k: np.asarray(v, dtype=np.float32) for k, v in inputs.items()}
    in_maps = host_prep(**inputs)
    res = run_bass_kernel_spmd(nc, in_maps, core_ids=list(range(8)))
    s, hid = inputs["hidden_states"].shape[1], inputs["hidden_states"].shape[2]
    out = np.zeros((B, s, hid), np.float32)
    for b in range(B):
        acc = np.zeros((s, hid), np.float64)
        for g in range(NKV):
            acc += res.results[b * NKV + g]["out"]
        out[b] = acc.astype(np.float32)
    return out


# revision 60
# speedup vs baseline: 1.2075x; 1.1318x over previous
"""Trainium2 Bass kernel for nn_AfmoeAttention (GQA attention + gated output).

Sharding: 8 cores = 2 batches x 4 kv-groups. Each core handles one batch and
one kv head with its 8 query heads (tensor-parallel over heads, o_proj
row-parallel with the partial sums reduced on host during unsharding).

Matmul strategy: fp8e4 DoubleRow (dual-fp8) where it pays. Each value is
carried as an fp8 pair (value8, residual8), so precision matches bf16 while
the PE runs at 0.5 cycles/row with 2x contraction per instruction:

  A:  q/k/v/gate projections: host supplies h (x8) as an fp8 pair (h8,dh8)
      laid out [p, kk, 2, s] and W (x32) as fp8 w8 + dw8. Per 128-chunk kk:
      instr1 = (h8+dh8)^T w8 (pair=(h8,dh8), moving w8 broadcast over the
      pair dim), and per chunk-pair: instr2 = h8^T dw8 (pair over two kk).
      24 DoubleRow matmuls per i-tile instead of 48 bf16-equivalents. The
      PSUM accumulators are evacuated to SBUF in three fast copies so the
      next i-tile's matmuls never wait (and the PE p-state never drops);
      ht loads ride the ACT DMA queue so the XBAR transposes on SP can't
      head-of-line block them. RMSNorm (scale-invariant, so the x256
      washes out) + RoPE run from the SBUF copy; q/k are emitted as
      [val8-as-bf16 | resid-bf16] staging, transposed via the bf16 DMA
      XBAR, then cast to fp8 packs post-transpose (casts on GpSimd, the
      last two i-tiles on ACT so phase B's opening key-tiles land early).
  B:  QK exact-fit: scores^T = (k8+dk8)^T (q8+dq8) in ONE DoubleRow matmul
      per (chunk, head, key-tile-pair) half: lhsT = [k8;k8 | dk8;dk8] pair
      blocks, moving = [q8T; dq8T] stacked partitions broadcast over the
      pair dim. 256 cycles per 128x512 score tile (2x the bf16 rate at
      bf16-level accuracy). Two key-tiles share one [128, 2, 512] PSUM
      tile (3 rotating tiles = 6 banks) and one 1024-wide exp, which
      alternates ScalarE AF.Exp (2/3) and DVE Schraudolph int16 (1/3).
      PV: P-stationary bf16 matmuls [keys,q128] x [v|1] -> PSUM [q, 65];
      col 64 accumulates the softmax denominator (x256 to cancel the
      projection scale). The pat accumulator is evacuated by one DVE copy
      (freeing its single PSUM bank for the next head) and the softmax
      normalization + gating run on the otherwise-idle GpSimd engine.
  C:  o_proj partial = gatedT^T @ WoT -> [s, HID] bf16 partials summed on
      host in fp64. Tiles pop two per head into the next chunk's slack
      (single PSUM bank; the drain after the last chunk reuses the freed
      QK PSUM ring to pipeline).
"""

import sys

import numpy as np

try:
    import concourse.bass as bass  # noqa: F401
except ImportError:
    sys.path.insert(0, "/opt/trn_rl_repo")

import concourse.mybir as mybir
import concourse.tile as tile
from concourse import bacc
from concourse.bass_utils import run_bass_kernel_spmd
from concourse.masks import make_identity  # noqa: F401  (identity kept for debug)


B, S, HID = 2, 2048, 2048
NH, NKV, D = 32, 4, 64
N_REP = NH // NKV            # 8 q-heads per kv head
EPS = 1e-6
SCALE = float(D) ** -0.5

# host-side pre-scales (compensated on device; RMSNorm is scale-invariant)
H_SC = 8.0                   # hidden_states scale before fp8 pairing
W_SC = 32.0                  # qkvg weight scale before fp8 pairing
HW_SC = H_SC * W_SC          # projection outputs are x256
Q_SC = 8.0                   # q/k scale baked into rope tables (fp8 range)

P = 128
FP32 = mybir.dt.float32
FP8 = mybir.dt.float8e4
BF16 = mybir.dt.bfloat16
I16 = mybir.dt.int16
AX = mybir.AxisListType.X
AF = mybir.ActivationFunctionType
DR = mybir.MatmulPerfMode.DoubleRow

# exp alternates strictly between ScalarE (AF.Exp) and the DVE (Schraudolph
# int16 bit-trick): consecutive units always overlap on different engines
# int16 Schraudolph constants: i = score * (ESC * 128*log2(e)) + B16,
# truncated to int16, bitcast bf16 ~= exp(score * ESC) with ~2% ripple
ESC = SCALE / (Q_SC * Q_SC)  # scores in PSUM are x64
A16 = 128.0 * 1.4426950408889634
B16 = 16256.0 - 5.5907


I32 = mybir.dt.int32
MAGIC = 0x5F3759DF
OP = mybir.AluOpType


def _rsqrt_dve(nc, stp, red, n, tag):
    """y = 1/sqrt(red/D + eps) on the DVE (bit-trick init + 1 Newton step...
    plus a refinement); scale-invariant use: red is x65536, y comes out /256.
    """
    x = stp.tile([P, n], FP32, tag=tag + "x")
    nc.vector.tensor_scalar(out=x[:], in0=red[:], scalar1=1.0 / D, scalar2=EPS,
                            op0=OP.mult, op1=OP.add)
    y = stp.tile([P, n], FP32, tag=tag + "y")
    nc.vector.tensor_scalar(out=y[:].bitcast(I32), in0=x[:].bitcast(I32),
                            scalar1=1, scalar2=None, op0=OP.arith_shift_right)
    nc.vector.tensor_scalar(out=y[:].bitcast(I32), in0=y[:].bitcast(I32),
                            scalar1=MAGIC, scalar2=-1,
                            op0=OP.subtract, op1=OP.mult)
    h = stp.tile([P, n], FP32, tag=tag + "h")
    nc.vector.tensor_scalar(out=h[:], in0=red[:], scalar1=0.5 / D, scalar2=0.5 * EPS,
                            op0=OP.mult, op1=OP.add)
    t = stp.tile([P, n], FP32, tag=tag + "t")
    for _ in range(1):
        nc.vector.tensor_mul(t[:], y[:], y[:])
        nc.vector.tensor_mul(t[:], t[:], h[:])
        nc.vector.tensor_scalar(out=t[:], in0=t[:], scalar1=-1.0, scalar2=1.5,
                                op0=OP.mult, op1=OP.add)
        nc.vector.tensor_mul(y[:], y[:], t[:])
    return y


def build_program(s=S, hid=HID):
    """Build and bacc-compile the single-core SPMD program."""
    KK = hid // P            # contraction tiles over HID
    KJ = KK // 2             # kk-pairs for the dW residual pass
    NI = s // P              # s-tiles
    SC = s // 512            # 512-wide s-chunks
    NC_HID = hid // 512      # o_proj output chunks
    PAIRS = N_REP // 2       # head pairs (o_proj lhsT blocks)
    F = N_REP * D            # 512: per-core q/gate feature width
    NW = F + 2 * D + F       # 1152: [q 512 | k 64 | v 64 | g 512]
    H2 = D // 2

    nc = bacc.Bacc("TRN2", target_bir_lowering=False, debug=False,
                   enable_asserts=True, num_devices=1)

    ht_d = nc.dram_tensor("ht", [s // P, P, KK * 2 * P], FP8, kind="ExternalInput")
    w8_d = nc.dram_tensor("w8", [P, KK, NW], FP8, kind="ExternalInput")
    dw8_d = nc.dram_tensor("dw8", [P, KK, NW], FP8, kind="ExternalInput")
    wot_d = nc.dram_tensor("wot", [F, hid], BF16, kind="ExternalInput")
    cq_d = nc.dram_tensor("cq", [s, D], FP32, kind="ExternalInput")
    sq_d = nc.dram_tensor("sq", [s, D], FP32, kind="ExternalInput")
    ck_d = nc.dram_tensor("ck", [s, D], FP32, kind="ExternalInput")
    sk_d = nc.dram_tensor("sk", [s, D], FP32, kind="ExternalInput")
    out_d = nc.dram_tensor("out", [s, hid], BF16, kind="ExternalOutput")

    wot_v = wot_d.ap().rearrange("(ft p) n -> p ft n", p=P)

    with tile.TileContext(nc) as tc:
        with tc.tile_pool(name="pers", bufs=1) as pers:
            # persistent across phases
            qpk = pers.tile([P, N_REP, NI, P], FP8, tag="qpk")   # [q8T;dq8T] stacks
            kpk = pers.tile([P, NI, 2, P], FP8, tag="kpk")       # pair blocks
            ve = pers.tile([P, NI, D + 1], BF16, tag="ve")       # [v | 1]
            sg = pers.tile([P, NI, F], BF16, tag="sg")           # sigmoid(gate)

            # the ones column carries HW_SC so the softmax-denominator
            # reciprocal also cancels the x256 projection scale on v
            nc.gpsimd.memset(ve[:, :, D:D + 1], HW_SC)

            # ---------------- Phase A: projections ----------------
            with tc.tile_pool(name="wq", bufs=1) as wqp, \
                 tc.tile_pool(name="ht", bufs=5) as htp, \
                 tc.tile_pool(name="cs", bufs=2) as csp, \
                 tc.tile_pool(name="scr", bufs=2) as scr, \
                 tc.tile_pool(name="stg", bufs=2) as stg, \
                 tc.tile_pool(name="stats", bufs=2) as stp, \
                 tc.tile_pool(name="psa", bufs=2, space="PSUM") as psa:

                w_sb = [None] * KK
                dw_sb = [None] * KK

                def get_w(kk):
                    # paired loads halve the HWDGE fixed overhead in warmup
                    j = kk // 2
                    if w_sb[j] is None:
                        wt = wqp.tile([P, 2, NW], FP8, name="wt", tag="w%d" % j)
                        nc.sync.dma_start(wt[:], w8_d.ap()[:, 2 * j:2 * j + 2, :])
                        w_sb[j] = wt
                    return w_sb[j][:, kk % 2, :]

                def get_dw(kk):
                    # loaded pairwise so instr2's [kk, kk+1] slice is one tile
                    j = kk // 2
                    if dw_sb[j] is None:
                        wt = wqp.tile([P, 2, NW], FP8, name="dwt", tag="dw%d" % j)
                        nc.sync.dma_start(wt[:], dw8_d.ap()[:, 2 * j:2 * j + 2, :])
                        dw_sb[j] = wt
                    return dw_sb[j]

                htq = {}

                def load_htb(i):
                    if i not in htq:
                        htb = htp.tile([P, KK, 2, P], FP8, name="htb", tag="hta")
                        nc.scalar.dma_start(
                            htb[:].rearrange("p a b c -> p (a b c)"),
                            ht_d.ap()[i, :, :])
                        htq[i] = htb
                    return htq[i]

                def load_ht(i):
                    htb = load_htb(i)
                    pq = psa.tile([P, F], FP32, name="pq", tag="pq")
                    pkv = psa.tile([P, 256], FP32, name="pkv", tag="pkv")
                    pg1 = psa.tile([P, 384], FP32, name="pg1", tag="pg1")
                    return (htb, pq, pkv, pg1)

                def emit_mms(st, kk):
                    # instr1: (h8+dh8)^T w8 ; pair dim on lhsT, w8 broadcast
                    htb, pq, pkv, pg1 = st
                    hp = htb[:, kk, :, :]
                    wt = get_w(kk)

                    def wbc(c0, c1):
                        return wt[:, c0:c1][:, None, :].broadcast_to(
                            [P, 2, c1 - c0])
                    nc.tensor.matmul(pq[:], hp, wbc(0, F),
                                     start=(kk == 0), stop=False, perf_mode=DR)
                    nc.tensor.matmul(pkv[:], hp, wbc(F, F + 256),
                                     start=(kk == 0), stop=False, perf_mode=DR)
                    nc.tensor.matmul(pg1[:], hp, wbc(F + 256, NW),
                                     start=(kk == 0), stop=False, perf_mode=DR)

                def emit_mms2(st, j):
                    # instr2: h8^T dw8 ; pair dim over (kk=2j, 2j+1)
                    htb, pq, pkv, pg1 = st
                    hp = htb[:, 2 * j:2 * j + 2, 0, :]
                    dwt = get_dw(2 * j)
                    last = (j == KJ - 1)
                    nc.tensor.matmul(pq[:], hp, dwt[:, :, 0:F],
                                     start=False, stop=last, perf_mode=DR)
                    nc.tensor.matmul(pkv[:], hp, dwt[:, :, F:F + 256],
                                     start=False, stop=last, perf_mode=DR)
                    nc.tensor.matmul(pg1[:], hp, dwt[:, :, F + 256:NW],
                                     start=False, stop=last, perf_mode=DR)

                def emit_all(st):
                    for kk in range(KK):
                        emit_mms(st, kk)
                    for j in range(KJ):
                        emit_mms2(st, j)

                warm = {}
                for i in range(NI):
                    if i == 0:
                        # interleave the first two iterations' matmuls so the
                        # PE does 6 MMs (not 3) per weight-tile arrival during
                        # the DMA-paced warmup; ht + weight DMAs are queued
                        # before the (2MB of) rope tables so the first matmul
                        # isn't stuck behind table traffic
                        st0 = load_ht(0)
                        st1 = load_ht(1)
                        for kk in range(KK):
                            get_w(kk)
                        load_htb(2)
                        cqa = csp.tile([P, NI, D], FP32, tag="cqa", bufs=1)
                        sqa = csp.tile([P, NI, D], FP32, tag="sqa", bufs=1)
                        cka = csp.tile([P, NI, D], FP32, tag="cka", bufs=1)
                        ska = csp.tile([P, NI, D], FP32, tag="ska", bufs=1)
                        cs_v = [t_d.ap().rearrange("(i p) d -> p i d", p=P)
                                for t_d in (cq_d, sq_d, ck_d, sk_d)]
                        nc.sync.dma_start(cqa[:], cs_v[0])
                        nc.sync.dma_start(sqa[:], cs_v[1])
                        nc.sync.dma_start(cka[:], cs_v[2])
                        nc.sync.dma_start(ska[:], cs_v[3])
                        for kk in range(0, KK, 2):
                            get_dw(kk)
                        load_htb(3)
                        load_htb(4)
                        for kk in range(KK):
                            emit_mms(st0, kk)
                            emit_mms(st1, kk)
                        for j in range(KJ):
                            emit_mms2(st0, j)
                            emit_mms2(st1, j)
                        warm[1] = st1
                        _, pq, pkv, pg1 = st0
                    elif i == 1:
                        _, pq, pkv, pg1 = warm.pop(1)
                    else:
                        st = load_ht(i)
                        if i + 1 < NI:
                            load_htb(i + 1)
                        if i + 2 < NI:
                            load_htb(i + 2)
                        emit_all(st)
                        _, pq, pkv, pg1 = st

                    cqt = cqa[:, i:i + 1, :]          # [P, 1, D]
                    sqt = sqa[:, i:i + 1, :]
                    ckt = cka[:, i, :]                # [P, D]
                    skt = ska[:, i, :]

                    # ---- stage 0: evacuate the PSUM accumulators to
                    # SBUF in three fast copies (split over DVE+ACT) so the
                    # psa WAR frees long before the PE drains the next
                    # i-tile's matmul queue; everything below reads the copy
                    ev = scr.tile([P, NW], FP32, tag="ev")
                    nc.vector.tensor_copy(ev[:, 0:F], pq[:])
                    nc.scalar.copy(ev[:, F:F + 256], pkv[:])
                    nc.scalar.copy(ev[:, F + 256:NW], pg1[:])
                    evq = ev[:, 0:F]
                    evk = ev[:, F:F + D]
                    evv = ev[:, F + D:F + 2 * D]
                    evg = ev[:, F + 2 * D:NW]  # [g0 128 | g1 384] halves

                    q3 = evq.rearrange("p (h d) -> p h d", d=D)
                    tsq = scr.tile([P, F], FP32, tag="tsq")
                    nc.scalar.square(tsq[:], evq)
                    red = stp.tile([P, N_REP + 1], FP32, tag="redq")
                    nc.vector.reduce_sum(red[:, 0:N_REP],
                                         tsq[:].rearrange("p (h d) -> p h d", d=D),
                                         axis=AX)
                    ktsq = scr.tile([P, D], FP32, tag="ktsq")
                    nc.scalar.activation(ktsq[:], evk, AF.Square,
                                         accum_out=red[:, N_REP:N_REP + 1])

                    # rope mixes (now SBUF reads, independent of the rsqrt)
                    kt2 = scr.tile([P, D], FP32, tag="kt2")
                    nc.vector.tensor_mul(kt2[:, 0:H2], evk[:, H2:D], skt[:, 0:H2])
                    nc.vector.tensor_mul(kt2[:, H2:D], evk[:, 0:H2], skt[:, H2:D])
                    kt3 = scr.tile([P, D], FP32, tag="kt3")
                    nc.vector.tensor_mul(kt3[:], evk, ckt[:])
                    nc.vector.tensor_add(kt3[:], kt3[:], kt2[:])
                    t2 = scr.tile([P, F], FP32, tag="t2")
                    t2v = t2[:].rearrange("p (h d) -> p h d", d=D)
                    nc.vector.tensor_mul(t2v[:, :, 0:H2], q3[:, :, H2:D],
                                         sqt[:, :, 0:H2].broadcast_to([P, N_REP, H2]))
                    nc.vector.tensor_mul(t2v[:, :, H2:D], q3[:, :, 0:H2],
                                         sqt[:, :, H2:D].broadcast_to([P, N_REP, H2]))
                    t3 = scr.tile([P, F], FP32, tag="t3")
                    t3v = t3[:].rearrange("p (h d) -> p h d", d=D)
                    nc.vector.tensor_mul(t3v, q3, cqt.broadcast_to([P, N_REP, D]))
                    nc.vector.tensor_add(t3[:], t3[:], t2[:])

                    # v + gate (SBUF now: v on Pool, tanh stays on ACT)
                    nc.gpsimd.tensor_copy(ve[:, i, 0:D], evv)
                    th = scr.tile([P, F], FP32, tag="th")
                    nc.scalar.activation(th[:, 0:P], evg[:, 0:P], AF.Tanh,
                                         scale=0.5 / HW_SC)
                    nc.scalar.activation(th[:, P:F], evg[:, P:F], AF.Tanh,
                                         scale=0.5 / HW_SC)

                    rall = _rsqrt_dve(nc, stp, red, N_REP + 1, "rq")
                    rms2 = rall[:, 0:N_REP]
                    krms2 = rall[:, N_REP:N_REP + 1]

                    late = i >= NI - 2   # Pool FIFO backlog would gate phase B

                    # ---- stage 2 (SBUF only): k pair staging -> XBAR first
                    # (phase B's opening units need every kpk tile)
                    kf = scr.tile([P, D], FP32, tag="kf")
                    nc.vector.tensor_mul(kf[:], kt3[:],
                                         krms2[:].broadcast_to([P, D]))
                    with nc.allow_low_precision(reason="fp8 pair k for DoubleRow QK"):
                        k8t = stg.tile([P, D], FP8, tag="k8t")
                        nc.scalar.copy(k8t[:], kf[:])
                        dk = stg.tile([P, D], BF16, tag="dk")
                        nc.vector.tensor_sub(dk[:], kf[:], k8t[:])
                        ksb = stg.tile([P, 2, 2, D], BF16, tag="ksb")
                        if late:
                            nc.scalar.copy(ksb[:, 0, :, :],
                                           k8t[:, None, :].broadcast_to([P, 2, D]))
                            nc.scalar.copy(ksb[:, 1, :, :],
                                           dk[:, None, :].broadcast_to([P, 2, D]))
                        else:
                            nc.gpsimd.tensor_copy(
                                ksb[:, 0, :, :],
                                k8t[:, None, :].broadcast_to([P, 2, D]))
                            nc.gpsimd.tensor_copy(
                                ksb[:, 1, :, :],
                                dk[:, None, :].broadcast_to([P, 2, D]))
                        kTs = stg.tile([P, 2, P], BF16, tag="kTs")
                        nc.sync.dma_start(
                            kTs[:], ksb[:].rearrange("p a b d -> p (a b d)"),
                            transpose=True)
                        if late:
                            nc.scalar.copy(kpk[:, i, :, :], kTs[:])
                        else:
                            nc.gpsimd.tensor_copy(kpk[:, i, :, :], kTs[:])

                    # ---- q pair staging -> XBAR
                    t1f = scr.tile([P, F], FP32, tag="t1f")
                    t1v = t1f[:].rearrange("p (h d) -> p h d", d=D)
                    nc.vector.tensor_mul(t1v, t3v, rms2[:, :, None].broadcast_to([P, N_REP, D]))
                    with nc.allow_low_precision(reason="fp8 pair q for DoubleRow QK"):
                        q8t = stg.tile([P, F], FP8, tag="q8t")
                        nc.scalar.copy(q8t[:], t1f[:])
                        qsb = stg.tile([P, N_REP, 2, D], BF16, tag="qsb")
                        nc.scalar.copy(
                            qsb[:, :, 0, :],
                            q8t[:].rearrange("p (h d) -> p h d", d=D))
                        nc.vector.tensor_sub(
                            qsb[:, :, 1, :],
                            t1v, q8t[:].rearrange("p (h d) -> p h d", d=D))
                        qTs = stg.tile([P, N_REP, P], BF16, tag="qTs")
                        nc.sync.dma_start(
                            qTs[:], qsb[:].rearrange("p h a d -> p (h a d)"),
                            transpose=True)
                        nc.gpsimd.tensor_copy(qpk[:, :, i, :], qTs[:])

                    # ---- sigmoid affine -> sg bf16 (SBUF, Pool)
                    with nc.allow_low_precision(reason="sigmoid affine to bf16 gate"):
                        nc.gpsimd.tensor_scalar(out=sg[:, i, :], in0=th[:],
                                                scalar1=0.5, scalar2=0.5,
                                                op0=OP.mult, op1=OP.add)

            # ---------------- Phase B: attention (+C overlapped) ----------------
            with tc.tile_pool(name="psqk", bufs=3, space="PSUM") as psqk, \
                 tc.tile_pool(name="psat", bufs=1, space="PSUM") as psat, \
                 tc.tile_pool(name="psc", bufs=1, space="PSUM") as psc, \
                 tc.tile_pool(name="expp", bufs=6) as expp, \
                 tc.tile_pool(name="misc", bufs=2) as mscp, \
                 tc.tile_pool(name="gst", bufs=4) as gsp, \
                 tc.tile_pool(name="wo", bufs=1) as wop:

                wot_sb = wop.tile([P, PAIRS, hid], BF16, tag="wot")
                nc.sync.dma_start(wot_sb[:], wot_v[:])

                qpk_v = qpk[:].rearrange("p h i s -> p h (i s)")

                cstate = {}

                def emit_c(gT, i, n, drain=False):
                    # one o_proj output tile [128, 512]; single PSUM slot in
                    # steady state (filler work), psqk's freed slots for the
                    # drain so back-to-back tiles pipeline there
                    if drain:
                        po = psqk.tile([P, 2, 512], FP32, name="pod",
                                       tag="pq4")[:, 0, :]
                    else:
                        po = psc.tile([P, 512], FP32, name="po", tag="po")
                    for ft in range(PAIRS):
                        nc.tensor.matmul(po[:], gT[:, ft, i % 4, :],
                                         wot_sb[:, ft, 512 * n:512 * (n + 1)],
                                         start=(ft == 0), stop=(ft == PAIRS - 1))
                    ob = mscp.tile([P, 512], BF16, name="ob", tag="ob", bufs=12)
                    with nc.allow_low_precision(reason="bf16 o_proj partials"):
                        if drain:
                            nc.scalar.copy(ob[:], po[:])  # ACT is idle post-exp
                        else:
                            nc.vector.tensor_copy(ob[:], po[:])
                    nc.sync.dma_start(
                        out_d.ap()[P * i:P * (i + 1), 512 * n:512 * (n + 1)], ob[:])

                pending = []
                NTP = NI // 2
                units = [(c, h, tp) for c in range(SC)
                         for h in range(N_REP) for tp in range(NTP)]
                ustate = {}
                pstate = {}

                def emit_qk(k):
                    c, h, tp = units[k]
                    pq4 = psqk.tile([P, 2, 512], FP32, tag="pq4")
                    rhs = qpk_v[:, h, 512 * c:512 * (c + 1)][:, None, :] \
                        .broadcast_to([P, 2, 512])
                    for dt in range(2):
                        nc.tensor.matmul(pq4[:, dt, :], kpk[:, 2 * tp + dt, :, :],
                                         rhs, start=True, stop=True, perf_mode=DR)
                    ustate[k] = pq4

                def emit_exp(k):
                    c, h, tp = units[k]
                    pq4 = ustate.pop(k)
                    pqf = pq4[:].rearrange("p a b -> p (a b)")
                    if k % 3 == 2:
                        # Schraudolph exp on the DVE: int16 bit-trick
                        e16 = expp.tile([P, 1024], I16, tag="e16", bufs=6)
                        with nc.allow_low_precision(reason="approx exp bit-trick"):
                            nc.vector.tensor_scalar(
                                out=e16[:], in0=pqf,
                                scalar1=ESC * A16, scalar2=B16,
                                op0=OP.mult, op1=OP.add)
                        ustate[k] = e16[:].bitcast(BF16).rearrange(
                            "p (a b) -> p a b", a=2)
                    else:
                        ex = expp.tile([P, 2, 512], BF16, tag="ee", bufs=6)
                        nc.scalar.activation(ex[:].rearrange("p a b -> p (a b)"),
                                             pqf, AF.Exp, scale=ESC)
                        ustate[k] = ex[:]

                def emit_pv(k):
                    c, h, tp = units[k]
                    expE = ustate.pop(k)
                    if tp == 0 and h == 0:
                        pstate['gstage'] = gsp.tile([P, 4, F], BF16, tag="gstage",
                                                    name="gstage")
                    if tp == 0:
                        # single tag: the pool's WAR dependency on the gating
                        # read of the previous head orders reuse correctly
                        pstate['pat'] = psat.tile([P, 4, D + 1], FP32,
                                                  tag="pat", name="pat")
                    pat = pstate['pat']
                    for dt in range(2):
                        for j in range(4):
                            # one accumulation group per pat tile: PSUM
                            # start/stop is zero-region (2KB) granular, so the
                            # 4 q-subtile accumulators share a single group
                            nc.tensor.matmul(
                                pat[:, j, :],
                                expE[:, dt, P * j:P * (j + 1)],
                                ve[:, 2 * tp + dt, :],
                                start=(tp == 0 and dt == 0 and j == 0),
                                stop=(tp == NTP - 1 and dt == 1 and j == 3),
                                skip_group_check=True)
                    if tp < NTP - 1:
                        return
                    # head done: one fast DVE copy evacuates pat (freeing
                    # the PSUM WAR for the next head's PV almost immediately),
                    # then the softmax normalization + gating run on the
                    # otherwise-idle Pool engine from SBUF
                    gstage = pstate['gstage']
                    pse = mscp.tile([P, 4, D + 1], FP32, tag="pse", bufs=2)
                    nc.vector.tensor_copy(pse[:], pat[:])
                    rr = mscp.tile([P, 4], FP32, tag="rr")
                    nc.vector.reciprocal(rr[:], pse[:, :, D])
                    fc = D * h
                    gtmp = mscp.tile([P, 4, D], FP32, tag="gtmp", bufs=2)
                    with nc.allow_low_precision(reason="gated attn bf16"):
                        nc.gpsimd.tensor_mul(
                            gtmp[:], pse[:, :, 0:D],
                            sg[:, 4 * c:4 * c + 4, fc:fc + D])
                        nc.gpsimd.tensor_mul(
                            gstage[:, :, fc:fc + D], gtmp[:],
                            rr[:, :, None].broadcast_to([P, 4, D]))
                    if h % 2 == 1:
                        # head pair done: transpose its gated [s, 128] block
                        # to [128, s] right away, so o_proj (and the final
                        # drain) never wait on the whole chunk's gating
                        if h == 1:
                            pstate['gT'] = gsp.tile([P, PAIRS, 4, P], BF16,
                                                    tag="gT", name="gT")
                        ft = h // 2
                        for j in range(4):
                            nc.sync.dma_start(
                                pstate['gT'][:, ft, j, :],
                                gstage[:, j, P * ft:P * (ft + 1)],
                                transpose=True)
                    if h < N_REP - 1:
                        return
                    gT = pstate['gT']
                    pending.extend((gT, 4 * c + j, n)
                                   for j in range(4) for n in range(NC_HID))

                # three decoupled streams: QK runs one pair-unit ahead of exp,
                # PV two behind, so neither ACT nor the PE wait-queue ever
                # blocks on an in-flight producer
                emit_qk(0)
                for k in range(len(units)):
                    emit_exp(k)
                    if k + 1 < len(units):
                        emit_qk(k + 1)
                    # o_proj AFTER the next QK so it never delays the exp
                    # stream's producer; 16 pop slots per chunk exactly match
                    # the 16 queued tiles, and tp==0/4 placement gives the PE
                    # filler work while the DVE runs the previous head's
                    # gating tail (pat WAR would otherwise idle the PE)
                    if pending and units[k][2] in (1, 5):
                        emit_c(*pending.pop(0))
                    if k >= 4:
                        emit_pv(k - 4)
                for k in range(len(units) - 4, len(units)):
                    emit_pv(k)

                # drain the last chunk's o_proj over rotating slots
                for (gT, i, n) in pending:
                    emit_c(gT, i, n, drain=True)

    nc.compile()
    return nc


def host_prep(hidden_states, cos, sin, Wq, Wk, Wv, Wg, Wo, q_gamma, k_gamma):
    """Shard and lay out the full inputs for the 8 cores (core = b*4 + g)."""
    import ml_dtypes
    bf = ml_dtypes.bfloat16
    f8 = ml_dtypes.float8_e4m3
    f = N_REP * D
    NW = f + 2 * D + f
    in_maps = []
    s = hidden_states.shape[1]
    KK = HID // P
    # hT pair-split: tile [i, p, kk, 2, c] with (h8, dh8), h scaled x8
    hT = []
    for b in range(B):
        x = np.asarray(hidden_states[b], dtype=np.float32) * H_SC
        t = x.reshape(s // P, P, KK, P)            # [i, c, kk, p]
        t = np.ascontiguousarray(t.transpose(0, 3, 2, 1))  # [i, p, kk, c]
        h8 = t.astype(f8)
        dh8 = (t - h8.astype(np.float32)).astype(f8)
        pk = np.stack([h8, dh8], axis=3)           # [i, p, kk, 2, c]
        hT.append(np.ascontiguousarray(pk.reshape(s // P, P, KK * 2 * P)))
    # sign pattern of rotate_half and the (permuted) gamma baked into sin/cos;
    # also the x8 fp8-range scale for q/k
    sgn = np.concatenate([-np.ones(D // 2, np.float32), np.ones(D // 2, np.float32)])
    gq_perm = np.roll(q_gamma, -(D // 2))
    gk_perm = np.roll(k_gamma, -(D // 2))
    tabs = []
    for b in range(B):
        cq = np.ascontiguousarray(cos[b] * q_gamma[None, :] * Q_SC).astype(np.float32)
        sq = np.ascontiguousarray(sin[b] * (sgn * gq_perm)[None, :] * Q_SC).astype(np.float32)
        ck = np.ascontiguousarray(cos[b] * k_gamma[None, :] * Q_SC).astype(np.float32)
        sk2 = np.ascontiguousarray(sin[b] * (sgn * gk_perm)[None, :] * Q_SC).astype(np.float32)
        tabs.append((cq, sq, ck, sk2))
    for b in range(B):
        for g in range(NKV):
            wq = Wq[f * g:f * (g + 1), :].T               # [hid, 512]
            wk = Wk[D * g:D * (g + 1), :].T               # [hid, 64]
            wv = Wv[D * g:D * (g + 1), :].T               # [hid, 64]
            wg_ = Wg[f * g:f * (g + 1), :].T              # [hid, 512]
            w = np.concatenate([wq, wk, wv, wg_], axis=1).astype(np.float32) * W_SC
            # [p, kk, nw] layout with hid split as (kk, p)
            wt = np.ascontiguousarray(
                w.reshape(KK, P, NW).transpose(1, 0, 2))
            w8 = wt.astype(f8)
            dw8 = (wt - w8.astype(np.float32)).astype(f8)
            wot = np.ascontiguousarray(Wo[:, f * g:f * (g + 1)].T).astype(bf)
            cq, sq, ck, sk2 = tabs[b]
            in_maps.append(dict(ht=hT[b], w8=np.ascontiguousarray(w8),
                                dw8=np.ascontiguousarray(dw8),
                                wot=wot, cq=cq, sq=sq, ck=ck, sk=sk2))
    return in_maps


_PROGRAM = None


def kernel(**inputs):
    global _PROGRAM
    if _PROGRAM is None:
        _PROGRAM = build_program()
    nc = _PROGRAM
    inputs = {k: np.asarray(v, dtype=np.float32) for k, v in inputs.items()}
    in_maps = host_prep(**inputs)
    res = run_bass_kernel_spmd(nc, in_maps, core_ids=list(range(8)))
    s, hid = inputs["hidden_states"].shape[1], inputs["hidden_states"].shape[2]
    out = np.zeros((B, s, hid), np.float32)
    for b in range(B):
        acc = np.zeros((s, hid), np.float64)
        for g in range(NKV):
            acc += res.results[b * NKV + g]["out"]
        out[b] = acc.astype(np.float32)
    return out


# revision 61
# speedup vs baseline: 1.2228x; 1.0127x over previous
"""Trainium2 Bass kernel for nn_AfmoeAttention (GQA attention + gated output).

Sharding: 8 cores = 2 batches x 4 kv-groups. Each core handles one batch and
one kv head with its 8 query heads (tensor-parallel over heads, o_proj
row-parallel with the partial sums reduced on host during unsharding).

Matmul strategy: fp8e4 DoubleRow (dual-fp8) where it pays. Each value is
carried as an fp8 pair (value8, residual8), so precision matches bf16 while
the PE runs at 0.5 cycles/row with 2x contraction per instruction:

  A:  q/k/v/gate projections: host supplies h (x8) as an fp8 pair (h8,dh8)
      laid out [p, kk, 2, s] and W (x32) as fp8 w8 + dw8. Per 128-chunk kk:
      instr1 = (h8+dh8)^T w8 (pair=(h8,dh8), moving w8 broadcast over the
      pair dim), and per chunk-pair: instr2 = h8^T dw8 (pair over two kk).
      24 DoubleRow matmuls per i-tile instead of 48 bf16-equivalents. The
      PSUM accumulators are evacuated to SBUF in three fast copies so the
      next i-tile's matmuls never wait (and the PE p-state never drops);
      ht loads ride the ACT DMA queue so the XBAR transposes on SP can't
      head-of-line block them. RMSNorm (scale-invariant, so the x256
      washes out) + RoPE run from the SBUF copy; q/k are emitted as
      [val8-as-bf16 | resid-bf16] staging, transposed via the bf16 DMA
      XBAR, then cast to fp8 packs post-transpose (casts on GpSimd, the
      last two i-tiles on ACT so phase B's opening key-tiles land early).
  B:  QK exact-fit: scores^T = (k8+dk8)^T (q8+dq8) in ONE DoubleRow matmul
      per (chunk, head, key-tile-pair) half: lhsT = [k8;k8 | dk8;dk8] pair
      blocks, moving = [q8T; dq8T] stacked partitions broadcast over the
      pair dim. 256 cycles per 128x512 score tile (2x the bf16 rate at
      bf16-level accuracy). Two key-tiles share one [128, 2, 512] PSUM
      tile (3 rotating tiles = 6 banks) and one 1024-wide exp, which
      alternates ScalarE AF.Exp (2/3) and DVE Schraudolph int16 (1/3).
      PV: P-stationary bf16 matmuls [keys,q128] x [v|1] -> PSUM [q, 65];
      col 64 accumulates the softmax denominator (x256 to cancel the
      projection scale). The pat accumulator is evacuated by one DVE copy
      (freeing its single PSUM bank for the next head) and the softmax
      normalization + gating run on the otherwise-idle GpSimd engine.
  C:  o_proj partial = gatedT^T @ WoT -> [s, HID] bf16 partials summed on
      host in fp64. Tiles pop two per head into the next chunk's slack
      (single PSUM bank; the drain after the last chunk reuses the freed
      QK PSUM ring to pipeline).
"""

import sys

import numpy as np

try:
    import concourse.bass as bass  # noqa: F401
except ImportError:
    sys.path.insert(0, "/opt/trn_rl_repo")

import concourse.mybir as mybir
import concourse.tile as tile
from concourse import bacc
from concourse.bass_utils import run_bass_kernel_spmd
from concourse.masks import make_identity  # noqa: F401  (identity kept for debug)


B, S, HID = 2, 2048, 2048
NH, NKV, D = 32, 4, 64
N_REP = NH // NKV            # 8 q-heads per kv head
EPS = 1e-6
SCALE = float(D) ** -0.5

# host-side pre-scales (compensated on device; RMSNorm is scale-invariant)
H_SC = 8.0                   # hidden_states scale before fp8 pairing
W_SC = 32.0                  # qkvg weight scale before fp8 pairing
HW_SC = H_SC * W_SC          # projection outputs are x256
Q_SC = 8.0                   # q/k scale baked into rope tables (fp8 range)

P = 128
FP32 = mybir.dt.float32
FP8 = mybir.dt.float8e4
BF16 = mybir.dt.bfloat16
I16 = mybir.dt.int16
AX = mybir.AxisListType.X
AF = mybir.ActivationFunctionType
DR = mybir.MatmulPerfMode.DoubleRow

# exp alternates strictly between ScalarE (AF.Exp) and the DVE (Schraudolph
# int16 bit-trick): consecutive units always overlap on different engines
# int16 Schraudolph constants: i = score * (ESC * 128*log2(e)) + B16,
# truncated to int16, bitcast bf16 ~= exp(score * ESC) with ~2% ripple
ESC = SCALE / (Q_SC * Q_SC)  # scores in PSUM are x64
A16 = 128.0 * 1.4426950408889634
B16 = 16256.0 - 5.5907


I32 = mybir.dt.int32
MAGIC = 0x5F3759DF
OP = mybir.AluOpType


def _rsqrt_dve(nc, stp, red, n, tag):
    """y = 1/sqrt(red/D + eps) on the DVE (bit-trick init + 1 Newton step...
    plus a refinement); scale-invariant use: red is x65536, y comes out /256.
    """
    x = stp.tile([P, n], FP32, tag=tag + "x")
    nc.vector.tensor_scalar(out=x[:], in0=red[:], scalar1=1.0 / D, scalar2=EPS,
                            op0=OP.mult, op1=OP.add)
    y = stp.tile([P, n], FP32, tag=tag + "y")
    nc.vector.tensor_scalar(out=y[:].bitcast(I32), in0=x[:].bitcast(I32),
                            scalar1=1, scalar2=None, op0=OP.arith_shift_right)
    nc.vector.tensor_scalar(out=y[:].bitcast(I32), in0=y[:].bitcast(I32),
                            scalar1=MAGIC, scalar2=-1,
                            op0=OP.subtract, op1=OP.mult)
    h = stp.tile([P, n], FP32, tag=tag + "h")
    nc.vector.tensor_scalar(out=h[:], in0=red[:], scalar1=0.5 / D, scalar2=0.5 * EPS,
                            op0=OP.mult, op1=OP.add)
    t = stp.tile([P, n], FP32, tag=tag + "t")
    for _ in range(1):
        nc.vector.tensor_mul(t[:], y[:], y[:])
        nc.vector.tensor_mul(t[:], t[:], h[:])
        nc.vector.tensor_scalar(out=t[:], in0=t[:], scalar1=-1.0, scalar2=1.5,
                                op0=OP.mult, op1=OP.add)
        nc.vector.tensor_mul(y[:], y[:], t[:])
    return y


def build_program(s=S, hid=HID):
    """Build and bacc-compile the single-core SPMD program."""
    KK = hid // P            # contraction tiles over HID
    KJ = KK // 2             # kk-pairs for the dW residual pass
    NI = s // P              # s-tiles
    SC = s // 512            # 512-wide s-chunks
    NC_HID = hid // 512      # o_proj output chunks
    PAIRS = N_REP // 2       # head pairs (o_proj lhsT blocks)
    F = N_REP * D            # 512: per-core q/gate feature width
    NW = F + 2 * D + F       # 1152: [q 512 | k 64 | v 64 | g 512]
    H2 = D // 2

    nc = bacc.Bacc("TRN2", target_bir_lowering=False, debug=False,
                   enable_asserts=True, num_devices=1)

    ht_d = nc.dram_tensor("ht", [s // P, P, KK * 2 * P], FP8, kind="ExternalInput")
    w8_d = nc.dram_tensor("w8", [P, KK, NW], FP8, kind="ExternalInput")
    dw8_d = nc.dram_tensor("dw8", [P, KK, NW], FP8, kind="ExternalInput")
    wot_d = nc.dram_tensor("wot", [F, hid], BF16, kind="ExternalInput")
    cq_d = nc.dram_tensor("cq", [s, D], FP32, kind="ExternalInput")
    sq_d = nc.dram_tensor("sq", [s, D], FP32, kind="ExternalInput")
    ck_d = nc.dram_tensor("ck", [s, D], FP32, kind="ExternalInput")
    sk_d = nc.dram_tensor("sk", [s, D], FP32, kind="ExternalInput")
    out_d = nc.dram_tensor("out", [s, hid], BF16, kind="ExternalOutput")

    wot_v = wot_d.ap().rearrange("(ft p) n -> p ft n", p=P)

    with tile.TileContext(nc) as tc:
        with tc.tile_pool(name="pers", bufs=1) as pers:
            # persistent across phases
            qpk = pers.tile([P, N_REP, NI, P], FP8, tag="qpk")   # [q8T;dq8T] stacks
            kpk = pers.tile([P, NI, 2, P], FP8, tag="kpk")       # pair blocks
            ve = pers.tile([P, NI, D + 1], BF16, tag="ve")       # [v | 1]
            sg = pers.tile([P, NI, F], BF16, tag="sg")           # sigmoid(gate)

            # the ones column carries HW_SC so the softmax-denominator
            # reciprocal also cancels the x256 projection scale on v
            nc.gpsimd.memset(ve[:, :, D:D + 1], HW_SC)

            # ---------------- Phase A: projections ----------------
            with tc.tile_pool(name="wq", bufs=1) as wqp, \
                 tc.tile_pool(name="ht", bufs=5) as htp, \
                 tc.tile_pool(name="cs", bufs=2) as csp, \
                 tc.tile_pool(name="scr", bufs=2) as scr, \
                 tc.tile_pool(name="stg", bufs=2) as stg, \
                 tc.tile_pool(name="stats", bufs=2) as stp, \
                 tc.tile_pool(name="psa", bufs=2, space="PSUM") as psa:

                w_sb = [None] * KK
                dw_sb = [None] * KK

                def get_w(kk):
                    # paired loads halve the HWDGE fixed overhead in warmup
                    j = kk // 2
                    if w_sb[j] is None:
                        wt = wqp.tile([P, 2, NW], FP8, name="wt", tag="w%d" % j)
                        nc.sync.dma_start(wt[:], w8_d.ap()[:, 2 * j:2 * j + 2, :])
                        w_sb[j] = wt
                    return w_sb[j][:, kk % 2, :]

                def get_dw(kk):
                    # loaded pairwise so instr2's [kk, kk+1] slice is one tile
                    j = kk // 2
                    if dw_sb[j] is None:
                        wt = wqp.tile([P, 2, NW], FP8, name="dwt", tag="dw%d" % j)
                        nc.sync.dma_start(wt[:], dw8_d.ap()[:, 2 * j:2 * j + 2, :])
                        dw_sb[j] = wt
                    return dw_sb[j]

                htq = {}

                def load_htb(i):
                    if i not in htq:
                        htb = htp.tile([P, KK, 2, P], FP8, name="htb", tag="hta")
                        nc.scalar.dma_start(
                            htb[:].rearrange("p a b c -> p (a b c)"),
                            ht_d.ap()[i, :, :])
                        htq[i] = htb
                    return htq[i]

                def load_ht(i):
                    htb = load_htb(i)
                    pq = psa.tile([P, F], FP32, name="pq", tag="pq")
                    pkv = psa.tile([P, 256], FP32, name="pkv", tag="pkv")
                    pg1 = psa.tile([P, 384], FP32, name="pg1", tag="pg1")
                    return (htb, pq, pkv, pg1)

                def emit_mms(st, kk):
                    # instr1: (h8+dh8)^T w8 ; pair dim on lhsT, w8 broadcast
                    htb, pq, pkv, pg1 = st
                    hp = htb[:, kk, :, :]
                    wt = get_w(kk)

                    def wbc(c0, c1):
                        return wt[:, c0:c1][:, None, :].broadcast_to(
                            [P, 2, c1 - c0])
                    nc.tensor.matmul(pq[:], hp, wbc(0, F),
                                     start=(kk == 0), stop=False, perf_mode=DR)
                    nc.tensor.matmul(pkv[:], hp, wbc(F, F + 256),
                                     start=(kk == 0), stop=False, perf_mode=DR)
                    nc.tensor.matmul(pg1[:], hp, wbc(F + 256, NW),
                                     start=(kk == 0), stop=False, perf_mode=DR)

                def emit_mms2(st, j):
                    # instr2: h8^T dw8 ; pair dim over (kk=2j, 2j+1)
                    htb, pq, pkv, pg1 = st
                    hp = htb[:, 2 * j:2 * j + 2, 0, :]
                    dwt = get_dw(2 * j)
                    last = (j == KJ - 1)
                    nc.tensor.matmul(pq[:], hp, dwt[:, :, 0:F],
                                     start=False, stop=last, perf_mode=DR)
                    nc.tensor.matmul(pkv[:], hp, dwt[:, :, F:F + 256],
                                     start=False, stop=last, perf_mode=DR)
                    nc.tensor.matmul(pg1[:], hp, dwt[:, :, F + 256:NW],
                                     start=False, stop=last, perf_mode=DR)

                def emit_all(st):
                    for kk in range(KK):
                        emit_mms(st, kk)
                    for j in range(KJ):
                        emit_mms2(st, j)

                warm = {}
                for i in range(NI):
                    if i == 0:
                        # interleave the first two iterations' matmuls so the
                        # PE does 6 MMs (not 3) per weight-tile arrival during
                        # the DMA-paced warmup; ht + weight DMAs are queued
                        # before the (2MB of) rope tables so the first matmul
                        # isn't stuck behind table traffic
                        st0 = load_ht(0)
                        st1 = load_ht(1)
                        for kk in range(KK):
                            get_w(kk)
                        load_htb(2)
                        cqa = csp.tile([P, NI, D], FP32, tag="cqa", bufs=1)
                        sqa = csp.tile([P, NI, D], FP32, tag="sqa", bufs=1)
                        cka = csp.tile([P, NI, D], FP32, tag="cka", bufs=1)
                        ska = csp.tile([P, NI, D], FP32, tag="ska", bufs=1)
                        cs_v = [t_d.ap().rearrange("(i p) d -> p i d", p=P)
                                for t_d in (cq_d, sq_d, ck_d, sk_d)]
                        nc.sync.dma_start(cqa[:], cs_v[0])
                        nc.sync.dma_start(sqa[:], cs_v[1])
                        nc.sync.dma_start(cka[:], cs_v[2])
                        nc.sync.dma_start(ska[:], cs_v[3])
                        for kk in range(0, KK, 2):
                            get_dw(kk)
                        load_htb(3)
                        load_htb(4)
                        for kk in range(KK):
                            emit_mms(st0, kk)
                            emit_mms(st1, kk)
                        for j in range(KJ):
                            emit_mms2(st0, j)
                            emit_mms2(st1, j)
                        warm[1] = st1
                        _, pq, pkv, pg1 = st0
                    elif i == 1:
                        _, pq, pkv, pg1 = warm.pop(1)
                    else:
                        st = load_ht(i)
                        if i + 1 < NI:
                            load_htb(i + 1)
                        if i + 2 < NI:
                            load_htb(i + 2)
                        emit_all(st)
                        _, pq, pkv, pg1 = st

                    cqt = cqa[:, i:i + 1, :]          # [P, 1, D]
                    sqt = sqa[:, i:i + 1, :]
                    ckt = cka[:, i, :]                # [P, D]
                    skt = ska[:, i, :]

                    # ---- stage 0: evacuate the PSUM accumulators to
                    # SBUF in three fast copies (split over DVE+ACT) so the
                    # psa WAR frees long before the PE drains the next
                    # i-tile's matmul queue; everything below reads the copy
                    ev = scr.tile([P, NW], FP32, tag="ev")
                    nc.vector.tensor_copy(ev[:, 0:F], pq[:])
                    nc.scalar.copy(ev[:, F:F + 256], pkv[:])
                    nc.scalar.copy(ev[:, F + 256:NW], pg1[:])
                    evq = ev[:, 0:F]
                    evk = ev[:, F:F + D]
                    evv = ev[:, F + D:F + 2 * D]
                    evg = ev[:, F + 2 * D:NW]  # [g0 128 | g1 384] halves

                    q3 = evq.rearrange("p (h d) -> p h d", d=D)
                    tsq = scr.tile([P, F], FP32, tag="tsq")
                    nc.scalar.square(tsq[:], evq)
                    red = stp.tile([P, N_REP + 1], FP32, tag="redq")
                    nc.vector.reduce_sum(red[:, 0:N_REP],
                                         tsq[:].rearrange("p (h d) -> p h d", d=D),
                                         axis=AX)
                    ktsq = scr.tile([P, D], FP32, tag="ktsq")
                    nc.scalar.activation(ktsq[:], evk, AF.Square,
                                         accum_out=red[:, N_REP:N_REP + 1])

                    # rope mixes (now SBUF reads, independent of the rsqrt)
                    kt2 = scr.tile([P, D], FP32, tag="kt2")
                    nc.vector.tensor_mul(kt2[:, 0:H2], evk[:, H2:D], skt[:, 0:H2])
                    nc.vector.tensor_mul(kt2[:, H2:D], evk[:, 0:H2], skt[:, H2:D])
                    kt3 = scr.tile([P, D], FP32, tag="kt3")
                    nc.vector.tensor_mul(kt3[:], evk, ckt[:])
                    nc.vector.tensor_add(kt3[:], kt3[:], kt2[:])
                    t2 = scr.tile([P, F], FP32, tag="t2")
                    t2v = t2[:].rearrange("p (h d) -> p h d", d=D)
                    nc.vector.tensor_mul(t2v[:, :, 0:H2], q3[:, :, H2:D],
                                         sqt[:, :, 0:H2].broadcast_to([P, N_REP, H2]))
                    nc.vector.tensor_mul(t2v[:, :, H2:D], q3[:, :, 0:H2],
                                         sqt[:, :, H2:D].broadcast_to([P, N_REP, H2]))
                    t3 = scr.tile([P, F], FP32, tag="t3")
                    t3v = t3[:].rearrange("p (h d) -> p h d", d=D)
                    nc.vector.tensor_mul(t3v, q3, cqt.broadcast_to([P, N_REP, D]))
                    nc.vector.tensor_add(t3[:], t3[:], t2[:])

                    # v + gate (SBUF now: v on Pool, tanh stays on ACT)
                    nc.gpsimd.tensor_copy(ve[:, i, 0:D], evv)
                    th = scr.tile([P, F], FP32, tag="th")
                    nc.scalar.activation(th[:, 0:P], evg[:, 0:P], AF.Tanh,
                                         scale=0.5 / HW_SC)
                    nc.scalar.activation(th[:, P:F], evg[:, P:F], AF.Tanh,
                                         scale=0.5 / HW_SC)

                    rall = _rsqrt_dve(nc, stp, red, N_REP + 1, "rq")
                    rms2 = rall[:, 0:N_REP]
                    krms2 = rall[:, N_REP:N_REP + 1]

                    late = i >= NI - 2   # Pool FIFO backlog would gate phase B

                    # ---- stage 2 (SBUF only): k pair staging -> XBAR first
                    # (phase B's opening units need every kpk tile)
                    kf = scr.tile([P, D], FP32, tag="kf")
                    nc.vector.tensor_mul(kf[:], kt3[:],
                                         krms2[:].broadcast_to([P, D]))
                    with nc.allow_low_precision(reason="fp8 pair k for DoubleRow QK"):
                        k8t = stg.tile([P, D], FP8, tag="k8t")
                        nc.scalar.copy(k8t[:], kf[:])
                        dk = stg.tile([P, D], BF16, tag="dk")
                        nc.vector.tensor_sub(dk[:], kf[:], k8t[:])
                        ksb = stg.tile([P, 2, 2, D], BF16, tag="ksb")
                        if late:
                            nc.scalar.copy(ksb[:, 0, :, :],
                                           k8t[:, None, :].broadcast_to([P, 2, D]))
                            nc.scalar.copy(ksb[:, 1, :, :],
                                           dk[:, None, :].broadcast_to([P, 2, D]))
                        else:
                            nc.gpsimd.tensor_copy(
                                ksb[:, 0, :, :],
                                k8t[:, None, :].broadcast_to([P, 2, D]))
                            nc.gpsimd.tensor_copy(
                                ksb[:, 1, :, :],
                                dk[:, None, :].broadcast_to([P, 2, D]))
                        kTs = stg.tile([P, 2, P], BF16, tag="kTs")
                        nc.sync.dma_start(
                            kTs[:], ksb[:].rearrange("p a b d -> p (a b d)"),
                            transpose=True)
                        if late:
                            nc.scalar.copy(kpk[:, i, :, :], kTs[:])
                        else:
                            nc.gpsimd.tensor_copy(kpk[:, i, :, :], kTs[:])

                    # ---- q pair staging -> XBAR
                    t1f = scr.tile([P, F], FP32, tag="t1f")
                    t1v = t1f[:].rearrange("p (h d) -> p h d", d=D)
                    nc.vector.tensor_mul(t1v, t3v, rms2[:, :, None].broadcast_to([P, N_REP, D]))
                    with nc.allow_low_precision(reason="fp8 pair q for DoubleRow QK"):
                        q8t = stg.tile([P, F], FP8, tag="q8t")
                        nc.scalar.copy(q8t[:], t1f[:])
                        qsb = stg.tile([P, N_REP, 2, D], BF16, tag="qsb")
                        nc.scalar.copy(
                            qsb[:, :, 0, :],
                            q8t[:].rearrange("p (h d) -> p h d", d=D))
                        nc.vector.tensor_sub(
                            qsb[:, :, 1, :],
                            t1v, q8t[:].rearrange("p (h d) -> p h d", d=D))
                        qTs = stg.tile([P, N_REP, P], BF16, tag="qTs")
                        nc.sync.dma_start(
                            qTs[:], qsb[:].rearrange("p h a d -> p (h a d)"),
                            transpose=True)
                        nc.gpsimd.tensor_copy(qpk[:, :, i, :], qTs[:])

                    # ---- sigmoid affine -> sg bf16 (SBUF, Pool)
                    with nc.allow_low_precision(reason="sigmoid affine to bf16 gate"):
                        nc.gpsimd.tensor_scalar(out=sg[:, i, :], in0=th[:],
                                                scalar1=0.5, scalar2=0.5,
                                                op0=OP.mult, op1=OP.add)

            # ---------------- Phase B: attention (+C overlapped) ----------------
            with tc.tile_pool(name="psqk", bufs=3, space="PSUM") as psqk, \
                 tc.tile_pool(name="psat", bufs=1, space="PSUM") as psat, \
                 tc.tile_pool(name="psc", bufs=1, space="PSUM") as psc, \
                 tc.tile_pool(name="expp", bufs=6) as expp, \
                 tc.tile_pool(name="misc", bufs=2) as mscp, \
                 tc.tile_pool(name="gst", bufs=4) as gsp, \
                 tc.tile_pool(name="wo", bufs=1) as wop:

                wot_sb = wop.tile([P, PAIRS, hid], BF16, tag="wot")
                nc.sync.dma_start(wot_sb[:], wot_v[:])

                qpk_v = qpk[:].rearrange("p h i s -> p h (i s)")

                cstate = {}

                def emit_c(gT, i, n, drain=False):
                    # one o_proj output tile [128, 512]; single PSUM slot in
                    # steady state (filler work), psqk's freed slots for the
                    # drain so back-to-back tiles pipeline there
                    if drain:
                        po = psqk.tile([P, 2, 512], FP32, name="pod",
                                       tag="pq4")[:, 0, :]
                    else:
                        po = psc.tile([P, 512], FP32, name="po", tag="po")
                    for ft in range(PAIRS):
                        nc.tensor.matmul(po[:], gT[:, ft, i % 4, :],
                                         wot_sb[:, ft, 512 * n:512 * (n + 1)],
                                         start=(ft == 0), stop=(ft == PAIRS - 1))
                    ob = mscp.tile([P, 512], BF16, name="ob", tag="ob", bufs=12)
                    with nc.allow_low_precision(reason="bf16 o_proj partials"):
                        if drain:
                            nc.scalar.copy(ob[:], po[:])  # ACT is idle post-exp
                        else:
                            nc.vector.tensor_copy(ob[:], po[:])
                    nc.sync.dma_start(
                        out_d.ap()[P * i:P * (i + 1), 512 * n:512 * (n + 1)], ob[:])

                pending = []
                NTP = NI // 2
                units = [(c, h, tp) for c in range(SC)
                         for h in range(N_REP) for tp in range(NTP)]
                ustate = {}
                pstate = {}

                def emit_qk(k):
                    c, h, tp = units[k]
                    pq4 = psqk.tile([P, 2, 512], FP32, tag="pq4")
                    rhs = qpk_v[:, h, 512 * c:512 * (c + 1)][:, None, :] \
                        .broadcast_to([P, 2, 512])
                    for dt in range(2):
                        nc.tensor.matmul(pq4[:, dt, :], kpk[:, 2 * tp + dt, :, :],
                                         rhs, start=True, stop=True, perf_mode=DR)
                    ustate[k] = pq4

                def emit_exp(k):
                    c, h, tp = units[k]
                    pq4 = ustate.pop(k)
                    pqf = pq4[:].rearrange("p a b -> p (a b)")
                    if k % 3 == 2:
                        # Schraudolph exp on the DVE: int16 bit-trick
                        e16 = expp.tile([P, 1024], I16, tag="e16", bufs=6)
                        with nc.allow_low_precision(reason="approx exp bit-trick"):
                            nc.vector.tensor_scalar(
                                out=e16[:], in0=pqf,
                                scalar1=ESC * A16, scalar2=B16,
                                op0=OP.mult, op1=OP.add)
                        ustate[k] = e16[:].bitcast(BF16).rearrange(
                            "p (a b) -> p a b", a=2)
                    else:
                        ex = expp.tile([P, 2, 512], BF16, tag="ee", bufs=6)
                        nc.scalar.activation(ex[:].rearrange("p a b -> p (a b)"),
                                             pqf, AF.Exp, scale=ESC)
                        ustate[k] = ex[:]

                def emit_pv(k):
                    c, h, tp = units[k]
                    expE = ustate.pop(k)
                    if tp == 0 and h == 0:
                        pstate['gstage'] = gsp.tile([P, 4, F], BF16, tag="gstage",
                                                    name="gstage")
                    if tp == 0:
                        # single tag: the pool's WAR dependency on the gating
                        # read of the previous head orders reuse correctly
                        pstate['pat'] = psat.tile([P, 4, D + 1], FP32,
                                                  tag="pat", name="pat")
                    pat = pstate['pat']
                    for dt in range(2):
                        for j in range(4):
                            # one accumulation group per pat tile: PSUM
                            # start/stop is zero-region (2KB) granular, so the
                            # 4 q-subtile accumulators share a single group
                            nc.tensor.matmul(
                                pat[:, j, :],
                                expE[:, dt, P * j:P * (j + 1)],
                                ve[:, 2 * tp + dt, :],
                                start=(tp == 0 and dt == 0 and j == 0),
                                stop=(tp == NTP - 1 and dt == 1 and j == 3),
                                skip_group_check=True)
                    if tp < NTP - 1:
                        return
                    # head done: one fast DVE copy evacuates pat (freeing
                    # the PSUM WAR for the next head's PV almost immediately),
                    # then the softmax normalization + gating run on the
                    # otherwise-idle Pool engine from SBUF
                    gstage = pstate['gstage']
                    pse = mscp.tile([P, 4, D + 1], FP32, tag="pse", bufs=2)
                    nc.vector.tensor_copy(pse[:], pat[:])
                    rr = mscp.tile([P, 4], FP32, tag="rr")
                    nc.vector.reciprocal(rr[:], pse[:, :, D])
                    fc = D * h
                    gtmp = mscp.tile([P, 4, D], FP32, tag="gtmp", bufs=2)
                    with nc.allow_low_precision(reason="gated attn bf16"):
                        nc.gpsimd.tensor_mul(
                            gtmp[:], pse[:, :, 0:D],
                            sg[:, 4 * c:4 * c + 4, fc:fc + D])
                        nc.gpsimd.tensor_mul(
                            gstage[:, :, fc:fc + D], gtmp[:],
                            rr[:, :, None].broadcast_to([P, 4, D]))
                    if h % 2 == 1:
                        # head pair done: transpose its gated [s, 128] block
                        # to [128, s] right away, so o_proj (and the final
                        # drain) never wait on the whole chunk's gating
                        if h == 1:
                            pstate['gT'] = gsp.tile([P, PAIRS, 4, P], BF16,
                                                    tag="gT", name="gT")
                        ft = h // 2
                        for j in range(4):
                            nc.sync.dma_start(
                                pstate['gT'][:, ft, j, :],
                                gstage[:, j, P * ft:P * (ft + 1)],
                                transpose=True)
                    if h < N_REP - 1:
                        return
                    gT = pstate['gT']
                    pending.extend((gT, 4 * c + j, n)
                                   for j in range(4) for n in range(NC_HID))

                # three decoupled streams: QK runs one pair-unit ahead of exp,
                # PV two behind, so neither ACT nor the PE wait-queue ever
                # blocks on an in-flight producer
                emit_qk(0)
                for k in range(len(units)):
                    emit_exp(k)
                    if k + 1 < len(units):
                        emit_qk(k + 1)
                    # o_proj AFTER the next QK so it never delays the exp
                    # stream's producer; 16 pop slots per chunk exactly match
                    # the 16 queued tiles, and tp==0/4 placement gives the PE
                    # filler work while the DVE runs the previous head's
                    # gating tail (pat WAR would otherwise idle the PE)
                    if pending and units[k][2] in (2, 6):
                        emit_c(*pending.pop(0))
                    if k >= 4:
                        emit_pv(k - 4)
                for k in range(len(units) - 4, len(units)):
                    emit_pv(k)

                # drain the last chunk's o_proj over rotating slots
                for (gT, i, n) in pending:
                    emit_c(gT, i, n, drain=True)

    nc.compile()
    return nc


def host_prep(hidden_states, cos, sin, Wq, Wk, Wv, Wg, Wo, q_gamma, k_gamma):
    """Shard and lay out the full inputs for the 8 cores (core = b*4 + g)."""
    import ml_dtypes
    bf = ml_dtypes.bfloat16
    f8 = ml_dtypes.float8_e4m3
    f = N_REP * D
    NW = f + 2 * D + f
    in_maps = []
    s = hidden_states.shape[1]
    KK = HID // P
    # hT pair-split: tile [i, p, kk, 2, c] with (h8, dh8), h scaled x8
    hT = []
    for b in range(B):
        x = np.asarray(hidden_states[b], dtype=np.float32) * H_SC
        t = x.reshape(s // P, P, KK, P)            # [i, c, kk, p]
        t = np.ascontiguousarray(t.transpose(0, 3, 2, 1))  # [i, p, kk, c]
        h8 = t.astype(f8)
        dh8 = (t - h8.astype(np.float32)).astype(f8)
        pk = np.stack([h8, dh8], axis=3)           # [i, p, kk, 2, c]
        hT.append(np.ascontiguousarray(pk.reshape(s // P, P, KK * 2 * P)))
    # sign pattern of rotate_half and the (permuted) gamma baked into sin/cos;
    # also the x8 fp8-range scale for q/k
    sgn = np.concatenate([-np.ones(D // 2, np.float32), np.ones(D // 2, np.float32)])
    gq_perm = np.roll(q_gamma, -(D // 2))
    gk_perm = np.roll(k_gamma, -(D // 2))
    tabs = []
    for b in range(B):
        cq = np.ascontiguousarray(cos[b] * q_gamma[None, :] * Q_SC).astype(np.float32)
        sq = np.ascontiguousarray(sin[b] * (sgn * gq_perm)[None, :] * Q_SC).astype(np.float32)
        ck = np.ascontiguousarray(cos[b] * k_gamma[None, :] * Q_SC).astype(np.float32)
        sk2 = np.ascontiguousarray(sin[b] * (sgn * gk_perm)[None, :] * Q_SC).astype(np.float32)
        tabs.append((cq, sq, ck, sk2))
    for b in range(B):
        for g in range(NKV):
            wq = Wq[f * g:f * (g + 1), :].T               # [hid, 512]
            wk = Wk[D * g:D * (g + 1), :].T               # [hid, 64]
            wv = Wv[D * g:D * (g + 1), :].T               # [hid, 64]
            wg_ = Wg[f * g:f * (g + 1), :].T              # [hid, 512]
            w = np.concatenate([wq, wk, wv, wg_], axis=1).astype(np.float32) * W_SC
            # [p, kk, nw] layout with hid split as (kk, p)
            wt = np.ascontiguousarray(
                w.reshape(KK, P, NW).transpose(1, 0, 2))
            w8 = wt.astype(f8)
            dw8 = (wt - w8.astype(np.float32)).astype(f8)
            wot = np.ascontiguousarray(Wo[:, f * g:f * (g + 1)].T).astype(bf)
            cq, sq, ck, sk2 = tabs[b]
            in_maps.append(dict(ht=hT[b], w8=np.ascontiguousarray(w8),
                                dw8=np.ascontiguousarray(dw8),
                                wot=wot, cq=cq, sq=sq, ck=ck, sk=sk2))
    return in_maps


_PROGRAM = None


def kernel(**inputs):
    global _PROGRAM
    if _PROGRAM is None:
        _PROGRAM = build_program()
    nc = _PROGRAM
    inputs = {k: np.asarray(v, dtype=np.float32) for k, v in inputs.items()}
    in_maps = host_prep(**inputs)
    res = run_bass_kernel_spmd(nc, in_maps, core_ids=list(range(8)))
    s, hid = inputs["hidden_states"].shape[1], inputs["hidden_states"].shape[2]
    out = np.zeros((B, s, hid), np.float32)
    for b in range(B):
        acc = np.zeros((s, hid), np.float64)
        for g in range(NKV):
            acc += res.results[b * NKV + g]["out"]
        out[b] = acc.astype(np.float32)
    return out
